# revision 1
# baseline (speedup 1.0000x reference)
"""Trainium2 Bass kernel for nn_EmergentRiskMetrics.

Contract: kernel(**inputs) takes the FULL unsharded inputs (as produced by
setup_inputs()) and returns the FULL output (shape [8], float32).

Sharding: data-parallel over the time axis for the two rolling-window
correlation scans (each of the 8 cores owns 1024 contiguous window starts
plus a halo), with the full-sequence [A,A] reductions (covariance /
correlation scalars, top-eigenvalue via trace-normalized repeated squaring,
sign-concordance matmul) and the tiny MLP replicated on every core.

Device outputs are per-core partial scalars; the host only gathers them
(sums partial sums, applies the final scalar clips/divides) to assemble the
8 outputs.
"""

import numpy as np

T = 8192
A = 128
W20 = 20
W10 = 10
NC_N = 8
CHUNK = 1024            # window starts per core
XROWS = 1152            # rows of per-core x_chunk (9 x 128, incl. halo)
NBLK = XROWS // 128     # 9
R20 = 128 + W20 - 1     # 147
R10 = 128 + W10 - 1     # 137
N20 = T - W20           # 8172 rolling-20 windows
N10 = T - W10           # 8182 rolling-10 windows
OUT_SLOTS = 24

# out_vec slot layout
S_COUNT20, S_HIST10, S_RECENT10, S_CSSUM, S_CSFIRST, S_CSLAST, \
    S_SUMCORR, S_SUMABS, S_TRACE, S_PASUM, S_PAMAX, S_SEV, S_SSQ, \
    S_T3, S_T6, S_T9, S_T12 = range(17)

_PLAN = {}


def _build_masks():
    m20 = np.zeros((128, R20 + 128), np.float32)
    m10 = np.zeros((128, R10 + 128), np.float32)
    for j in range(128):
        m20[j, j:j + W20] = 1.0
        m20[j, R20 + j] = -1.0 / W20
        m10[j, j:j + W10] = 1.0
        m10[j, R10 + j] = -1.0 / W10
    return m20, m10


def _core_masks(c):
    g = c * CHUNK + np.arange(CHUNK)
    valid20 = (g < N20).astype(np.float32)
    hist10 = (g < N10 - 5).astype(np.float32)
    recent10 = ((g >= N10 - 5) & (g < N10)).astype(np.float32)
    # device layout [128 partitions, 8 chunk-columns]
    return (np.ascontiguousarray(valid20.reshape(8, 128).T),
            np.ascontiguousarray(hist10.reshape(8, 128).T),
            np.ascontiguousarray(recent10.reshape(8, 128).T))


def _build_program():
    import os
    import concourse.bacc as bacc
    import concourse.tile as tile
    from concourse import mybir

    kbits = int(os.environ.get("KBITS", "63"))
    rollpart = int(os.environ.get("ROLLPART", "3"))
    chunkdepth = int(os.environ.get("CHUNKDEPTH", "5"))
    DO_ROLL = kbits & 1
    DO_CS = kbits & 2
    DO_COV = kbits & 4
    DO_EIG = kbits & 8
    DO_POS = kbits & 16
    DO_MLP = kbits & 32

    f32 = mybir.dt.float32
    bf16 = mybir.dt.bfloat16
    ALU = mybir.AluOpType
    ACT = mybir.ActivationFunctionType
    AX = mybir.AxisListType

    nc = bacc.Bacc("TRN2", target_bir_lowering=False, debug=False,
                   num_devices=NC_N)

    def din(name, shape):
        return nc.dram_tensor(name, shape, f32, kind="ExternalInput").ap()

    x_full = din("x_full", [T, A])
    x_chunk = din("x_chunk", [XROWS, A])
    mask20 = din("mask20", [128, R20 + 128])
    mask10 = din("mask10", [128, R10 + 128])
    valid20 = din("valid20", [128, 8])
    hist10 = din("hist10", [128, 8])
    recent10 = din("recent10", [128, 8])
    ident_in = din("ident", [128, 128])
    w1a_in = din("w1a", [128, 128])
    w1b_in = din("w1b", [128, 128])
    b1_in = din("b1", [128, 1])
    gamma_in = din("gamma", [128, 1])
    beta_in = din("beta", [128, 1])
    w2_in = din("w2", [128, 64])
    b2_in = din("b2", [64, 1])
    w3_in = din("w3", [64, 3])
    b3_in = din("b3", [3, 1])
    pos_in = din("positions", [128, 1])
    xlast_in = din("xlast", [128, 1])
    oh127_in = din("onehot127", [128, 1])
    oh2_in = din("onehot2", [3, 1])
    out_d = nc.dram_tensor("out_vec", [1, OUT_SLOTS], f32,
                           kind="ExternalOutput").ap()

    with tile.TileContext(nc) as tc:
        with tc.tile_pool(name="const", bufs=1) as cst, \
             tc.tile_pool(name="persist", bufs=1) as per, \
             tc.tile_pool(name="xfs", bufs=8) as xfs, \
             tc.tile_pool(name="sgs", bufs=4) as sgs, \
             tc.tile_pool(name="work", bufs=4) as wrk, \
             tc.tile_pool(name="small", bufs=6) as sml, \
             tc.tile_pool(name="dram", bufs=1, space="DRAM") as dram, \
             tc.tile_pool(name="ps", bufs=1, space="PSUM") as ps:

            psum_bufs = {"covq": 1, "mq": 1, "zp": 2, "big": 2, "sc": 2}

            def psum(shape, tag):
                return ps.tile(shape, f32, tag=tag, name=tag,
                               bufs=psum_bufs[tag])

            # ---- constants ----
            ident = cst.tile([128, 128], f32, tag="ident")
            nc.sync.dma_start(ident[:], ident_in[:, :])
            m20 = cst.tile([128, R20 + 128], f32, tag="m20")
            nc.sync.dma_start(m20[:], mask20[:, :])
            m10 = cst.tile([128, R10 + 128], f32, tag="m10")
            nc.sync.dma_start(m10[:], mask10[:, :])
            v20 = cst.tile([128, 8], f32, tag="v20")
            nc.sync.dma_start(v20[:], valid20[:, :])
            h10 = cst.tile([128, 8], f32, tag="h10")
            nc.sync.dma_start(h10[:], hist10[:, :])
            r10 = cst.tile([128, 8], f32, tag="r10")
            nc.sync.dma_start(r10[:], recent10[:, :])
            ones = cst.tile([128, 1], f32, tag="ones")
            nc.vector.memset(ones[:], 1.0)
            ones_row = cst.tile([1, 128], f32, tag="ones_row")
            nc.vector.memset(ones_row[:], 1.0)
            oh127 = cst.tile([128, 1], f32, tag="oh127")
            nc.sync.dma_start(oh127[:], oh127_in[:, :])
            oh2 = cst.tile([3, 1], f32, tag="oh2")
            nc.sync.dma_start(oh2[:], oh2_in[:, :])

            out_sb = per.tile([1, OUT_SLOTS], f32, tag="out_sb")
            nc.vector.memset(out_sb[:], 0.0)

            def slot(i):
                return out_sb[:, i:i + 1]

            # sum over partitions of an SBUF [p,1] vector -> [1,1] psum
            def psum_scalar(vec_sb, p=128):
                o = psum([1, 1], "sc")
                lhs = ones[0:p, :] if p != 128 else ones[:]
                nc.tensor.matmul(o[:], lhsT=lhs, rhs=vec_sb,
                                 start=True, stop=True, skip_group_check=True)
                return o

            # ================= per-core rolling windows =================
            # load x_chunk tiles and transpose to xT [A, XROWS]
            if True:
                xT = per.tile([128, XROWS], f32, tag="xT")
            xcs = []
            for j in range(NBLK):
                xc = per.tile([128, 128], f32, tag="xc%d" % j, name="xc%d" % j)
                nc.sync.dma_start(xc[:], x_chunk[j * 128:(j + 1) * 128, :])
                xcs.append(xc)
                tp = psum([128, 128], "big")
                nc.tensor.transpose(tp[:], xc[:], ident[:])
                nc.vector.tensor_copy(xT[:, j * 128:(j + 1) * 128], tp[:])

            x2T = per.tile([128, XROWS], f32, tag="x2T")
            nc.scalar.activation(x2T[:], xT[:], ACT.Square)

            # rolling sums via log-shift adds (shared s2/s4/s8 for w=10/20)
            def rollsums(src, sums_tag, eng):
                s2 = per.tile([128, 1151], f32, tag=sums_tag + "s2")
                eng.tensor_add(s2[:], src[:, 0:1151], src[:, 1:1152])
                s4 = per.tile([128, 1149], f32, tag=sums_tag + "s4")
                eng.tensor_add(s4[:], s2[:, 0:1149], s2[:, 2:1151])
                s8 = per.tile([128, 1145], f32, tag=sums_tag + "s8")
                eng.tensor_add(s8[:], s4[:, 0:1145], s4[:, 4:1149])
                s16 = per.tile([128, 1137], f32, tag=sums_tag + "s16")
                eng.tensor_add(s16[:], s8[:, 0:1137], s8[:, 8:1145])
                s20 = per.tile([128, 1088], f32, tag=sums_tag + "s20")
                eng.tensor_add(s20[:], s16[:, 0:1088], s4[:, 16:1104])
                s10 = per.tile([128, 1088], f32, tag=sums_tag + "s10")
                eng.tensor_add(s10[:], s8[:, 0:1088], s2[:, 8:1096])
                return s20, s10

            S20, S10 = rollsums(xT, "S", nc.vector)
            P20, P10 = rollsums(x2T, "P", nc.gpsimd)

            onesw = cst.tile([128, CHUNK], f32, tag="onesw")
            nc.vector.memset(onesw[:], 1.0)

            def make_u(S, P, w, tag):
                sq = per.tile([128, CHUNK], f32, tag=tag + "sq")
                nc.gpsimd.tensor_mul(sq[:], S[:, 0:CHUNK], S[:, 0:CHUNK])
                d2 = per.tile([128, CHUNK], f32, tag=tag + "d2")
                nc.vector.scalar_tensor_tensor(
                    d2[:], in0=sq[:], scalar=-1.0 / w, in1=P[:, 0:CHUNK],
                    op0=ALU.mult, op1=ALU.add)
                d = per.tile([128, CHUNK], f32, tag=tag + "d")
                nc.scalar.activation(d[:], d2[:], ACT.Sqrt)
                u = per.tile([128, CHUNK], f32, tag=tag + "u")
                for kk in range(8):
                    nc.vector.reciprocal(u[:, kk * 128:(kk + 1) * 128],
                                         d[:, kk * 128:(kk + 1) * 128])
                return u

            if DO_ROLL:
                u20 = make_u(S20, P20, W20, "u20")
                u10 = make_u(S10, P10, W10, "u10")
                if rollpart == 1:
                    uu = sml.tile([128, 1], f32, tag="uu")
                    nc.vector.tensor_reduce(uu[:], u20[:, 0:8], axis=AX.X,
                                            op=ALU.add)
                    nc.vector.tensor_copy(slot(S_COUNT20),
                                          psum_scalar(uu[:])[:])

            count_acc = per.tile([128, 1], f32, tag="count_acc")
            nc.vector.memset(count_acc[:], 0.0)
            hist_acc = per.tile([128, 1], f32, tag="hist_acc")
            nc.vector.memset(hist_acc[:], 0.0)
            rec_acc = per.tile([128, 1], f32, tag="rec_acc")
            nc.vector.memset(rec_acc[:], 0.0)

            inv_od = 1.0 / (A * (A - 1))

            def roll_chunk(k, u, S, R, m):
                zp = psum([128, R + 128], "zp")
                nc.tensor.matmul(zp[:, 0:R],
                                 lhsT=u[:, k * 128:(k + 1) * 128],
                                 rhs=xT[:, k * 128:k * 128 + R],
                                 start=True, stop=True, skip_group_check=True)
                if chunkdepth == 1:
                    r1 = sml.tile([128, 1], f32, tag="r1")
                    nc.vector.tensor_reduce(r1[:], zp[:, 0:R], axis=AX.X,
                                            op=ALU.add)
                    return r1
                nc.tensor.matmul(zp[:, R:R + 128],
                                 lhsT=u[:, k * 128:(k + 1) * 128],
                                 rhs=S[:, k * 128:(k + 1) * 128],
                                 start=True, stop=True, skip_group_check=True)
                if chunkdepth == 2:
                    r1 = sml.tile([128, 1], f32, tag="r1")
                    nc.vector.tensor_reduce(r1[:], zp[:], axis=AX.X,
                                            op=ALU.add)
                    return r1
                V = wrk.tile([128, R + 128], f32, tag="V")
                nc.scalar.activation(V[:], zp[:], ACT.Square)
                if chunkdepth == 3:
                    r1 = sml.tile([128, 1], f32, tag="r1")
                    nc.vector.tensor_reduce(r1[:], V[:], axis=AX.X,
                                            op=ALU.add)
                    return r1
                scr = wrk.tile([128, R + 128], f32, tag="scr")
                nc.gpsimd.tensor_mul(scr[:], V[:], m[:])
                acc = sml.tile([128, 1], f32, tag="acc")
                nc.vector.tensor_reduce(acc[:], scr[:], axis=AX.X,
                                        op=ALU.add)
                if chunkdepth == 4:
                    return acc
                roll = sml.tile([128, 1], f32, tag="roll")
                nc.vector.tensor_scalar(roll[:], acc[:], -float(A), inv_od,
                                        ALU.add, ALU.mult)
                return roll

            for k in range(8 if (DO_ROLL and rollpart >= 2) else 0):
                roll = roll_chunk(k, u20, S20, R20, m20)
                if rollpart == 2:
                    nc.vector.tensor_add(count_acc[:], count_acc[:], roll[:])
                    continue
                cmp = sml.tile([128, 1], f32, tag="cmp")
                nc.vector.tensor_scalar(cmp[:], roll[:], 0.7, None, ALU.is_gt)
                cmp2 = sml.tile([128, 1], f32, tag="cmp2")
                nc.vector.tensor_mul(cmp2[:], cmp[:], v20[:, k:k + 1])
                nc.vector.tensor_add(count_acc[:], count_acc[:], cmp2[:])

                roll = roll_chunk(k, u10, S10, R10, m10)
                hv = sml.tile([128, 1], f32, tag="hv")
                nc.vector.tensor_mul(hv[:], roll[:], h10[:, k:k + 1])
                nc.vector.tensor_add(hist_acc[:], hist_acc[:], hv[:])
                rv = sml.tile([128, 1], f32, tag="rv")
                nc.vector.tensor_mul(rv[:], roll[:], r10[:, k:k + 1])
                nc.vector.tensor_add(rec_acc[:], rec_acc[:], rv[:])

            if DO_ROLL and rollpart >= 2:
                nc.vector.tensor_copy(slot(S_COUNT20),
                                      psum_scalar(count_acc[:])[:])
                nc.vector.tensor_copy(slot(S_HIST10),
                                      psum_scalar(hist_acc[:])[:])
                nc.vector.tensor_copy(slot(S_RECENT10),
                                      psum_scalar(rec_acc[:])[:])

            # ================= cross-sectional std (herding) =================
            csstd = per.tile([128, 8], f32, tag="csstd")
            for b in range(8 if DO_CS else 0):
                srow = psum([128, 1], "sc")
                nc.tensor.matmul(srow[:], lhsT=xT[:, b * 128:(b + 1) * 128],
                                 rhs=ones[:], start=True, stop=True,
                                 skip_group_check=True)
                ssq = psum([128, 1], "sc")
                nc.tensor.matmul(ssq[:], lhsT=x2T[:, b * 128:(b + 1) * 128],
                                 rhs=ones[:], start=True, stop=True,
                                 skip_group_check=True)
                sq = sml.tile([128, 1], f32, tag="cs_sq")
                nc.scalar.activation(sq[:], srow[:], ACT.Square)
                var = sml.tile([128, 1], f32, tag="cs_var")
                nc.vector.scalar_tensor_tensor(
                    var[:], in0=sq[:], scalar=-1.0 / A, in1=ssq[:],
                    op0=ALU.mult, op1=ALU.add)
                nc.scalar.activation(csstd[:, b:b + 1], var[:], ACT.Sqrt,
                                     scale=1.0 / (A - 1))
            if DO_CS:
                csr = sml.tile([128, 1], f32, tag="csr")
                nc.vector.tensor_reduce(csr[:], csstd[:], axis=AX.X,
                                        op=ALU.add)
                nc.vector.tensor_copy(slot(S_CSSUM), psum_scalar(csr[:])[:])
                nc.vector.tensor_copy(slot(S_CSFIRST), csstd[0:1, 0:1])
                cslast_p = psum([1, 1], "sc")
                nc.tensor.matmul(cslast_p[:], lhsT=oh127[:],
                                 rhs=csstd[:, 7:8],
                                 start=True, stop=True, skip_group_check=True)
                nc.vector.tensor_copy(slot(S_CSLAST), cslast_p[:])

            # ================= full-T covariance + sign concordance ========
            covq = psum([128, 128], "covq")
            mq = psum([128, 128], "mq")
            for i in range(64 if DO_COV else 0):
                xf = xfs.tile([128, 128], f32, tag="xf")
                nc.sync.dma_start(xf[:], x_full[i * 128:(i + 1) * 128, :])
                st, sp = (i == 0), (i == 63)
                nc.tensor.matmul(covq[:], lhsT=xf[:], rhs=xf[:],
                                 start=st, stop=sp, skip_group_check=True)
                sg = sgs.tile([128, 128], bf16, tag="sg")
                nc.scalar.activation(sg[:], xf[:], ACT.Sign)
                nc.tensor.matmul(mq[:], lhsT=sg[:], rhs=sg[:],
                                 start=st, stop=sp, skip_group_check=True)

            # ssq_sum = total sum of sign-concordance matmul
            if not DO_COV:
                corr = None
            if DO_COV:
                mr = sml.tile([128, 1], f32, tag="mr")
                nc.vector.tensor_reduce(mr[:], mq[:], axis=AX.X, op=ALU.add)
                nc.vector.tensor_copy(slot(S_SSQ), psum_scalar(mr[:])[:])

                # cov = Q - S S^T / T
                cov = per.tile([128, 128], f32, tag="cov")
                nc.vector.tensor_copy(cov[:], covq[:])

                # diag, u = 1/sqrt(diag)
                dscr = wrk.tile([128, 128], f32, tag="dscr")
                nc.vector.tensor_mul(dscr[:], cov[:], ident[:])
                diag = per.tile([128, 1], f32, tag="diag")
                nc.vector.tensor_reduce(diag[:], dscr[:], axis=AX.X,
                                        op=ALU.add)
                dstd = per.tile([128, 1], f32, tag="dstd")
                nc.scalar.activation(dstd[:], diag[:], ACT.Sqrt)
                ucol = per.tile([128, 1], f32, tag="ucol")
                nc.vector.reciprocal(ucol[:], dstd[:])
                # trace(corr) = sum diag * u * u
                u2 = sml.tile([128, 1], f32, tag="u2")
                nc.vector.tensor_mul(u2[:], ucol[:], ucol[:])
                du2 = sml.tile([128, 1], f32, tag="du2")
                nc.vector.tensor_mul(du2[:], u2[:], diag[:])
                nc.vector.tensor_copy(slot(S_TRACE), psum_scalar(du2[:])[:])

                # u^T as a row for the quadratic forms
                uT_p = psum([1, 128], "sc")
                nc.tensor.transpose(uT_p[:], ucol[:], ident[:])
                uT = per.tile([1, 128], f32, tag="uT")
                nc.vector.tensor_copy(uT[:], uT_p[:])

                def quad_form(mat_sb, out_slot):
                    qr = psum([1, 128], "sc")
                    nc.tensor.matmul(qr[:], lhsT=ucol[:], rhs=mat_sb,
                                     start=True, stop=True, skip_group_check=True)
                    qscr = sml.tile([1, 128], f32, tag="qscr")
                    nc.vector.tensor_mul(qscr[:], qr[:], uT[:])
                    qacc = sml.tile([1, 1], f32, tag="qacc")
                    nc.vector.tensor_reduce(qacc[:], qscr[:], axis=AX.X,
                                            op=ALU.add)
                    nc.vector.tensor_copy(out_slot, qacc[:])

                quad_form(cov[:], slot(S_SUMCORR))
                acov = per.tile([128, 128], f32, tag="acov")
                nc.scalar.activation(acov[:], cov[:], ACT.Abs)
                quad_form(acov[:], slot(S_SUMABS))

                # corr = diag(u) @ cov @ diag(u), via row-scale, transpose, row-scale
                brow = per.tile([128, 128], f32, tag="brow")
                nc.vector.tensor_scalar(brow[:], cov[:], ucol[:], None, ALU.mult)
                bt_p = psum([128, 128], "big")
                nc.tensor.transpose(bt_p[:], brow[:], ident[:])
                corr = per.tile([128, 128], f32, tag="corr")
                nc.vector.tensor_scalar(corr[:], bt_p[:], ucol[:], None, ALU.mult)

            # ---- top eigenvalue: 12 squarings, trace-normalize at 3,6,9,12 --
            if DO_COV and DO_EIG:
                norm_slots = {2: S_T3, 5: S_T6, 8: S_T9}
                M = corr
                for kk in range(9):
                    p = psum([128, 128], "big")
                    nc.tensor.matmul(p[:], lhsT=M[:], rhs=M[:],
                                     start=True, stop=True, skip_group_check=True)
                    Mn = wrk.tile([128, 128], f32, tag="Mn")
                    if kk in norm_slots:
                        escr = wrk.tile([128, 128], f32, tag="escr")
                        nc.vector.tensor_mul(escr[:], p[:], ident[:])
                        edg = sml.tile([128, 1], f32, tag="edg")
                        nc.vector.tensor_reduce(edg[:], escr[:], axis=AX.X,
                                                op=ALU.add)
                        trp = psum_scalar(edg[:])
                        tr_sb = sml.tile([1, 1], f32, tag="tr_sb")
                        nc.vector.tensor_copy(tr_sb[:], trp[:])
                        nc.vector.tensor_copy(slot(norm_slots[kk]), tr_sb[:])
                        bc = psum([128, 1], "sc")
                        nc.tensor.matmul(bc[:], lhsT=ones_row[:], rhs=tr_sb[:],
                                         start=True, stop=True,
                                         skip_group_check=True)
                        rcp = sml.tile([128, 1], f32, tag="rcp")
                        nc.vector.reciprocal(rcp[:], bc[:])
                        nc.vector.tensor_scalar(Mn[:], p[:], rcp[:], None,
                                                ALU.mult)
                    else:
                        nc.vector.tensor_copy(Mn[:], p[:])
                    M = Mn

            # ================= position diversity =================
            pos_sb = per.tile([128, 1], f32, tag="pos_sb")
            nc.sync.dma_start(pos_sb[:], pos_in[:, :])
            if DO_POS:
                pa = per.tile([128, 1], f32, tag="pa")
                nc.scalar.activation(pa[:], pos_sb[:], ACT.Abs)
                nc.vector.tensor_copy(slot(S_PASUM), psum_scalar(pa[:])[:])
                paT_p = psum([1, 128], "sc")
                nc.tensor.transpose(paT_p[:], pa[:], ident[:])
                paT = sml.tile([1, 128], f32, tag="paT")
                nc.vector.tensor_copy(paT[:], paT_p[:])
                nc.vector.tensor_reduce(slot(S_PAMAX), paT[:], axis=AX.X,
                                        op=ALU.max)

            # ================= herding MLP =================
            if DO_MLP:
                w1a = cst.tile([128, 128], f32, tag="w1a")
                nc.sync.dma_start(w1a[:], w1a_in[:, :])
                w1b = cst.tile([128, 128], f32, tag="w1b")
                nc.sync.dma_start(w1b[:], w1b_in[:, :])
                b1 = cst.tile([128, 1], f32, tag="b1")
                nc.sync.dma_start(b1[:], b1_in[:, :])
                gam = cst.tile([128, 1], f32, tag="gam")
                nc.sync.dma_start(gam[:], gamma_in[:, :])
                bet = cst.tile([128, 1], f32, tag="bet")
                nc.sync.dma_start(bet[:], beta_in[:, :])
                w2 = cst.tile([128, 64], f32, tag="w2")
                nc.sync.dma_start(w2[:], w2_in[:, :])
                b2 = cst.tile([64, 1], f32, tag="b2")
                nc.sync.dma_start(b2[:], b2_in[:, :])
                w3 = cst.tile([64, 3], f32, tag="w3")
                nc.sync.dma_start(w3[:], w3_in[:, :])
                b3 = cst.tile([3, 1], f32, tag="b3")
                nc.sync.dma_start(b3[:], b3_in[:, :])
                xl = cst.tile([128, 1], f32, tag="xl")
                nc.sync.dma_start(xl[:], xlast_in[:, :])

                h1p = psum([128, 1], "sc")
                nc.tensor.matmul(h1p[:], lhsT=w1a[:], rhs=xl[:], start=True,
                                 stop=False, skip_group_check=True)
                nc.tensor.matmul(h1p[:], lhsT=w1b[:], rhs=pos_sb[:], start=False,
                                 stop=True, skip_group_check=True)
                h1 = sml.tile([128, 1], f32, tag="h1")
                nc.scalar.activation(h1[:], h1p[:], ACT.Relu, bias=b1[:])
                gk = sml.tile([128, 1], f32, tag="gk")
                nc.vector.tensor_scalar(gk[:], gam[:],
                                        float(1.0 / np.sqrt(1.0 + 1e-5)), None,
                                        ALU.mult)
                h1b = sml.tile([128, 1], f32, tag="h1b")
                nc.vector.tensor_scalar(h1b[:], h1[:], gk[:], bet[:],
                                        ALU.mult, ALU.add)
                h2p = psum([64, 1], "sc")
                nc.tensor.matmul(h2p[:], lhsT=w2[:], rhs=h1b[:], start=True,
                                 stop=True, skip_group_check=True)
                h2 = sml.tile([64, 1], f32, tag="h2")
                nc.scalar.activation(h2[:], h2p[:], ACT.Relu, bias=b2[:])
                lg = psum([3, 1], "sc")
                nc.tensor.matmul(lg[:], lhsT=w3[:], rhs=h2[:], start=True,
                                 stop=True, skip_group_check=True)
                exps = sml.tile([3, 1], f32, tag="exps")
                nc.scalar.activation(exps[:], lg[:], ACT.Exp, bias=b3[:])
                esum = psum_scalar(exps[:], p=3)
                esum_sb = sml.tile([1, 1], f32, tag="esum_sb")
                nc.vector.tensor_copy(esum_sb[:], esum[:])
                erec = sml.tile([1, 1], f32, tag="erec")
                nc.vector.reciprocal(erec[:], esum_sb[:])
                e2p = psum([1, 1], "sc")
                nc.tensor.matmul(e2p[:], lhsT=oh2[:], rhs=exps[:], start=True,
                                 stop=True, skip_group_check=True)
                e2_sb = sml.tile([1, 1], f32, tag="e2_sb")
                nc.vector.tensor_copy(e2_sb[:], e2p[:])
                nc.vector.tensor_mul(slot(S_SEV), e2_sb[:], erec[:])

            # ================= write out =================
            nc.sync.dma_start(out_d[:, :], out_sb[:])

    nc.compile()
    return nc


def _prep_in_maps(inputs):
    x = np.ascontiguousarray(np.asarray(inputs["returns_sequence"],
                                        dtype=np.float32))
    pos = np.asarray(inputs["positions"], dtype=np.float32).reshape(128, 1)
    m20, m10 = _build_masks()
    ident = np.eye(128, dtype=np.float32)
    w1 = np.asarray(inputs["w1"], np.float32)
    common = {
        "x_full": x,
        "mask20": m20, "mask10": m10, "ident": ident,
        "w1a": np.ascontiguousarray(w1[0:128]),
        "w1b": np.ascontiguousarray(w1[128:256]),
        "b1": np.asarray(inputs["b1"], np.float32).reshape(128, 1),
        "gamma": np.asarray(inputs["gamma"], np.float32).reshape(128, 1),
        "beta": np.asarray(inputs["beta"], np.float32).reshape(128, 1),
        "w2": np.asarray(inputs["w2"], np.float32),
        "b2": np.asarray(inputs["b2"], np.float32).reshape(64, 1),
        "w3": np.asarray(inputs["w3"], np.float32),
        "b3": np.asarray(inputs["b3"], np.float32).reshape(3, 1),
        "positions": pos,
        "xlast": np.ascontiguousarray(x[-1].reshape(128, 1)),
        "onehot127": np.eye(128, dtype=np.float32)[:, 127:128].copy(),
        "onehot2": np.array([[0.0], [0.0], [1.0]], np.float32),
    }
    in_maps = []
    for c in range(NC_N):
        rows = (c * CHUNK + np.arange(XROWS)) % T
        v20, h10, r10 = _core_masks(c)
        m = dict(common)
        m["x_chunk"] = np.ascontiguousarray(x[rows])
        m["valid20"] = v20
        m["hist10"] = h10
        m["recent10"] = r10
        in_maps.append(m)
    return in_maps


def _combine(per_core):
    g = per_core[0][0]  # replicated scalars from core 0
    count20 = sum(float(per_core[c][0, S_COUNT20]) for c in range(NC_N))
    hist_s = sum(float(per_core[c][0, S_HIST10]) for c in range(NC_N))
    rec_s = sum(float(per_core[c][0, S_RECENT10]) for c in range(NC_N))
    cs_sum = sum(float(per_core[c][0, S_CSSUM]) for c in range(NC_N))
    cs_first = float(per_core[0][0, S_CSFIRST])
    cs_last = float(per_core[NC_N - 1][0, S_CSLAST])
    r0 = per_core[0][0]
    sum_corr = float(r0[S_SUMCORR])
    sum_abs = float(r0[S_SUMABS])
    trace_c = float(r0[S_TRACE])
    pa_sum = float(r0[S_PASUM])
    pa_max = float(r0[S_PAMAX])
    severity = float(r0[S_SEV])
    ssq_sum = float(r0[S_SSQ])
    T3, T6, T9, T12 = (float(r0[S_T3]), float(r0[S_T6]),
                       float(r0[S_T9]), float(r0[S_T12]))

    phase_locking = count20 / N20
    hist = hist_s / (N10 - 5)
    recent = rec_s / 5.0
    surge = 0.0
    if hist > 0:
        surge = min(max((recent - hist) / hist, 0.0), 1.0)
    avg_disp = cs_sum / T
    trend = -(cs_last - cs_first) / (T - 1)
    herding_index = min(max(trend / (avg_disp + 1e-6) + 0.5, 0.0), 1.0)
    avg_corr = (sum_corr - trace_c) / (A * (A - 1))
    lam = np.exp((64.0 * np.log(T3) + 8.0 * np.log(T6) + np.log(T9)) / 512.0)
    sync_risk = min(1.0, (lam / A) * avg_corr)
    return_div = 1.0 - sum_abs / (A * A)
    pos_div = 1.0 - pa_max / pa_sum
    div_loss = 1.0 - np.sqrt(return_div * pos_div)
    avg_conc = (A * A / 2.0 + ssq_sum / (2.0 * T) - A) / (A * (A - 1))
    phase_coupling = min(max((avg_conc - 0.5) * 2.0, 0.0), 1.0)
    collective = (herding_index + sync_risk + div_loss) / 3.0
    return np.array([herding_index, severity, sync_risk, phase_locking,
                     div_loss, surge, phase_coupling, collective],
                    dtype=np.float32)


def _ensure_ntff_hook():
    """Install the axon NTFF profile hook if the image lacks antenv.axon_hooks."""
    import sys
    import types
    try:
        import antenv.axon_hooks  # noqa: F401
        return True
    except ImportError:
        pass
    try:
        import antenv
        from trn_agent_boot.trn_boot import _ntff_profile_via_ctypes
        mod = types.ModuleType("antenv.axon_hooks")
        state = {}
        mod.set_axon_ntff_profile_hook = lambda h: state.update(h=h)
        mod.get_axon_ntff_profile_hook = lambda: state.get("h")
        sys.modules["antenv.axon_hooks"] = mod
        antenv.axon_hooks = mod
        hook = _ntff_profile_via_ctypes("/opt/axon/libaxon_pjrt.so")
        mod.set_axon_ntff_profile_hook(hook)
        return hook is not None
    except Exception:
        return False


def _run(inputs, trace=False):
    from concourse.bass_utils import run_bass_kernel_spmd
    if trace:
        trace = _ensure_ntff_hook()
    if "nc" not in _PLAN:
        _PLAN["nc"] = _build_program()
    nc = _PLAN["nc"]
    in_maps = _prep_in_maps(inputs)
    res = run_bass_kernel_spmd(nc, in_maps, core_ids=list(range(NC_N)),
                               trace=trace)
    per_core = [res.results[c]["out_vec"] for c in range(NC_N)]
    return _combine(per_core), res


def kernel(**inputs) -> np.ndarray:
    out, _ = _run(inputs, trace=False)
    return out



# revision 5
# speedup vs baseline: 1.0097x; 1.0097x over previous
"""Trainium2 Bass kernel for nn_EmergentRiskMetrics.

Contract: kernel(**inputs) takes the FULL unsharded inputs (as produced by
setup_inputs()) and returns the FULL output (shape [8], float32).

Sharding: data-parallel over the time axis. Each of the 8 cores owns 1024
contiguous window starts (plus a 128-row halo) for the two rolling-window
correlation scans, and the same 1024 rows form its shard of the full-T
covariance / sign-concordance matmuls. The cov partials are AllReduced
across the 8 cores (64 KB); the [A,A] reductions (correlation scalars,
top-eigenvalue via repeated squaring) and the tiny MLP then run replicated.
Sign-concordance reduces to a per-core scalar that the host sums.

Windowed sums are computed with banded-matrix matmuls on the tensor engine
(bands pre-scaled by 1/sqrt(w) so the mean-correction q^2 term needs no
extra scaling), u = 1/std via reciprocal_approx_fast + Sqrt, and the whole
rolling path runs in bf16 (validated: all rolling-derived outputs have
orders-of-magnitude margin against bf16 noise).

Device outputs are per-core partial scalars; the host only gathers them
(sums partial sums, applies the final scalar clips/divides) to assemble the
8 outputs.
"""

import numpy as np

T = 8192
A = 128
W20 = 20
W10 = 10
NC_N = 8
CHUNK = 1024            # window starts per core
XROWS = 1152            # rows of per-core x_chunk (9 x 128, incl. halo)
NBLK = XROWS // 128     # 9
R20 = 128 + W20 - 1     # 147
R10 = 128 + W10 - 1     # 137
N20 = T - W20           # 8172 rolling-20 windows
N10 = T - W10           # 8182 rolling-10 windows
OUT_SLOTS = 24
INV_OD = 1.0 / (A * (A - 1))
# rolling20 > 0.7 in corr units == raw quadratic sum > this
THRESH20 = 0.7 * (A * (A - 1)) + A

S_COUNT20, S_HIST10, S_RECENT10, S_CSSUM, S_CSFIRST, S_CSLAST, \
    S_SUMCORR, S_SUMABS, S_TRACE, S_PASUM, S_PAMAX, S_SEV, S_SSQ, \
    S_T6, S_T9 = range(15)

_PLAN = {}


def _build_masks():
    # V-masks: +1 over the window band, -1 on the q^2 column
    m20 = np.zeros((128, R20 + 128), np.float32)
    m10 = np.zeros((128, R10 + 128), np.float32)
    for j in range(128):
        m20[j, j:j + W20] = 1.0
        m20[j, R20 + j] = -1.0
        m10[j, j:j + W10] = 1.0
        m10[j, R10 + j] = -1.0
    return m20, m10


def _build_bands():
    # bands0/1 [128 t, 256]: cols 0:128 window-20 (scaled 1/sqrt20),
    # cols 128:256 window-10 (scaled 1/sqrt10). S' = B0^T x_k + B1^T x_{k+1}
    b0 = np.zeros((128, 256), np.float32)
    b1 = np.zeros((128, 256), np.float32)
    s20 = 1.0 / np.sqrt(W20)
    s10 = 1.0 / np.sqrt(W10)
    for j in range(128):
        for t in range(128):
            if j <= t < j + W20:
                b0[t, j] = s20
            if j <= t + 128 < j + W20:
                b1[t, j] = s20
            if j <= t < j + W10:
                b0[t, 128 + j] = s10
            if j <= t + 128 < j + W10:
                b1[t, 128 + j] = s10
    return b0, b1


def _core_masks(c):
    g = c * CHUNK + np.arange(CHUNK)
    valid20 = (g < N20).astype(np.float32)
    hist10 = (g < N10 - 5).astype(np.float32)
    recent10 = ((g >= N10 - 5) & (g < N10)).astype(np.float32)
    # device layout [128 partitions (j in chunk), 8 chunk-columns]
    return (np.ascontiguousarray(valid20.reshape(8, 128).T),
            np.ascontiguousarray(hist10.reshape(8, 128).T),
            np.ascontiguousarray(recent10.reshape(8, 128).T))


def _build_program():
    import os
    import concourse.bacc as bacc
    import concourse.tile as tile
    from concourse import mybir

    kbits = int(os.environ.get("KBITS", "63"))
    DO_ROLL = kbits & 1
    DO_CS = kbits & 2
    DO_COV = kbits & 4
    DO_EIG = kbits & 8
    DO_POS = kbits & 16
    DO_MLP = kbits & 32

    f32 = mybir.dt.float32
    bf16 = mybir.dt.bfloat16
    ALU = mybir.AluOpType
    ACT = mybir.ActivationFunctionType
    AX = mybir.AxisListType

    nc = bacc.Bacc("TRN2", target_bir_lowering=False, debug=False,
                   num_devices=NC_N)

    def din(name, shape):
        return nc.dram_tensor(name, shape, f32, kind="ExternalInput").ap()

    x_chunk = din("x_chunk", [XROWS, A])
    bands0_in = din("bands0", [128, 256])
    bands1_in = din("bands1", [128, 256])
    mask20 = din("mask20", [128, R20 + 128])
    mask10 = din("mask10", [128, R10 + 128])
    valid20 = din("valid20", [128, 8])
    hist10 = din("hist10", [128, 8])
    recent10 = din("recent10", [128, 8])
    ident_in = din("ident", [128, 128])
    w1a_in = din("w1a", [128, 128])
    w1b_in = din("w1b", [128, 128])
    b1_in = din("b1", [128, 1])
    gamma_in = din("gamma", [128, 1])
    beta_in = din("beta", [128, 1])
    w2_in = din("w2", [128, 64])
    b2_in = din("b2", [64, 1])
    w3_in = din("w3", [64, 3])
    b3_in = din("b3", [3, 1])
    pos_in = din("positions", [128, 1])
    xlast_in = din("xlast", [128, 1])
    oh127_in = din("onehot127", [128, 1])
    oh2_in = din("onehot2", [3, 1])
    out_d = nc.dram_tensor("out_vec", [1, OUT_SLOTS], f32,
                           kind="ExternalOutput").ap()

    with tile.TileContext(nc) as tc:
        with tc.tile_pool(name="const", bufs=1) as cst, \
             tc.tile_pool(name="persist", bufs=1) as per, \
             tc.tile_pool(name="sgs", bufs=3) as sgs, \
             tc.tile_pool(name="wrk", bufs=3) as wrk, \
             tc.tile_pool(name="small", bufs=6) as sml, \
             tc.tile_pool(name="dram", bufs=1, space="DRAM") as dram, \
             tc.tile_pool(name="ps", bufs=1, space="PSUM") as ps:

            psum_bufs = {"covq": 1, "mq": 1, "band": 2, "zp": 2,
                         "big": 1, "sc": 1}

            def psum(shape, tag):
                return ps.tile(shape, f32, tag=tag, name=tag,
                               bufs=psum_bufs[tag])

            # ---- constants ----
            ident = cst.tile([128, 128], f32, tag="ident")
            nc.sync.dma_start(ident[:], ident_in[:, :])
            b0f = cst.tile([128, 256], f32, tag="b0f")
            nc.sync.dma_start(b0f[:], bands0_in[:, :])
            b1f = cst.tile([128, 256], f32, tag="b1f")
            nc.sync.dma_start(b1f[:], bands1_in[:, :])
            b0b = cst.tile([128, 256], bf16, tag="b0b")
            nc.vector.tensor_copy(b0b[:], b0f[:])
            b1b = cst.tile([128, 256], bf16, tag="b1b")
            nc.vector.tensor_copy(b1b[:], b1f[:])
            m20 = cst.tile([128, R20 + 128], f32, tag="m20")
            nc.sync.dma_start(m20[:], mask20[:, :])
            m10 = cst.tile([128, R10 + 128], f32, tag="m10")
            nc.sync.dma_start(m10[:], mask10[:, :])
            v20 = cst.tile([128, 8], f32, tag="v20")
            nc.sync.dma_start(v20[:], valid20[:, :])
            h10 = cst.tile([128, 8], f32, tag="h10")
            nc.sync.dma_start(h10[:], hist10[:, :])
            r10 = cst.tile([128, 8], f32, tag="r10")
            nc.sync.dma_start(r10[:], recent10[:, :])
            ones = cst.tile([128, 1], f32, tag="ones")
            nc.vector.memset(ones[:], 1.0)
            ones_row = cst.tile([1, 128], f32, tag="ones_row")
            nc.vector.memset(ones_row[:], 1.0)
            oh127 = cst.tile([128, 1], f32, tag="oh127")
            nc.sync.dma_start(oh127[:], oh127_in[:, :])
            oh2 = cst.tile([3, 1], f32, tag="oh2")
            nc.sync.dma_start(oh2[:], oh2_in[:, :])

            out_sb = per.tile([1, OUT_SLOTS], f32, tag="out_sb")
            nc.vector.memset(out_sb[:], 0.0)

            def slot(i):
                return out_sb[:, i:i + 1]

            # sum over partitions of an SBUF [p,1] vector -> [1,1] psum
            def psum_scalar(vec_sb, p=128):
                o = psum([1, 1], "sc")
                lhs = ones[0:p, :] if p != 128 else ones[:]
                nc.tensor.matmul(o[:], lhsT=lhs, rhs=vec_sb,
                                 start=True, stop=True, skip_group_check=True)
                return o

            # ================= tile loads + per-tile preps =================
            xcs, xcbs, xsqbs = [], [], []
            for j in range(NBLK):
                xc = per.tile([128, 128], f32, tag="xc%d" % j, name="xc%d" % j)
                nc.sync.dma_start(xc[:], x_chunk[j * 128:(j + 1) * 128, :])
                xcs.append(xc)
                xcb = per.tile([128, 128], bf16, tag="xcb%d" % j)
                nc.vector.tensor_copy(xcb[:], xc[:])
                xcbs.append(xcb)
                xsqb = per.tile([128, 128], bf16, tag="xsqb%d" % j)
                nc.gpsimd.tensor_mul(xsqb[:], xc[:], xc[:])
                xsqbs.append(xsqb)

            # ============ sharded cov + sign concordance (blocks 0..7) =====
            covq = psum([128, 128], "covq")
            mq = psum([128, 128], "mq")
            for i in range(8 if DO_COV else 0):
                st, sp = (i == 0), (i == 7)
                nc.tensor.matmul(covq[:], lhsT=xcbs[i][:], rhs=xcbs[i][:],
                                 start=st, stop=sp, skip_group_check=True)
                sg = sgs.tile([128, 128], bf16, tag="sg")
                nc.scalar.activation(sg[:], xcs[i][:], ACT.Sign)
                nc.tensor.matmul(mq[:], lhsT=sg[:], rhs=sg[:],
                                 start=st, stop=sp, skip_group_check=True)

            cov = per.tile([128, 128], f32, tag="cov")
            if DO_COV:
                # per-core partial of the total sign-concordance sum
                mr = sml.tile([128, 1], f32, tag="mr")
                nc.vector.tensor_reduce(mr[:], mq[:], axis=AX.X, op=ALU.add)
                nc.vector.tensor_copy(slot(S_SSQ), psum_scalar(mr[:])[:])

                # AllReduce cov partials across the 8 cores (DRAM bounce)
                covsb = per.tile([128, 128], f32, tag="covsb")
                nc.scalar.activation(covsb[:], covq[:], ACT.Copy)
                cin = dram.tile([128, 128], f32, tag="cin")
                cout = dram.tile([128, 128], f32, tag="cout")
                nc.gpsimd.dma_start(cin[:], covsb[:])
                nc.gpsimd.collective_compute(
                    "AllReduce", ALU.add,
                    replica_groups=[list(range(NC_N))],
                    ins=[cin.opt()], outs=[cout.opt()])
                nc.gpsimd.dma_start(cov[:], cout[:])

            # ================= transposes: xTb [A, XROWS] bf16 =============
            xTb = per.tile([128, XROWS], bf16, tag="xTb")
            for j in range(NBLK):
                tp = psum([128, 128], "big")
                nc.tensor.transpose(tp[:], xcs[j][:], ident[:])
                nc.vector.tensor_copy(xTb[:, j * 128:(j + 1) * 128], tp[:])

            # ================= rolling windows, 8 chunks ===================
            num20 = per.tile([128, 8], f32, tag="num20")
            num10 = per.tile([128, 8], f32, tag="num10")
            for k in range(8 if DO_ROLL else 0):
                # windowed sums S' (scaled by 1/sqrt w) and P
                sp_ = psum([128, 256], "band")
                nc.tensor.matmul(sp_[:], lhsT=xcbs[k][:], rhs=b0b[:],
                                 start=True, stop=False, skip_group_check=True)
                nc.tensor.matmul(sp_[:], lhsT=xcbs[k + 1][:], rhs=b1b[:],
                                 start=False, stop=True, skip_group_check=True)
                pp = psum([128, 256], "band")
                nc.tensor.matmul(pp[:], lhsT=xsqbs[k][:], rhs=b0b[:],
                                 start=True, stop=False, skip_group_check=True)
                nc.tensor.matmul(pp[:], lhsT=xsqbs[k + 1][:], rhs=b1b[:],
                                 start=False, stop=True, skip_group_check=True)
                # d2 = P - S^2/w = pp*sqrt(w) - S'^2 ; u = 1/sqrt(d2) in bf16
                # (gpsimd cannot read PSUM, so square S' on the scalar engine)
                sq = wrk.tile([128, 256], f32, tag="sq")
                nc.scalar.activation(sq[:], sp_[:], ACT.Square)
                d2 = wrk.tile([128, 256], f32, tag="d2")
                nc.vector.scalar_tensor_tensor(
                    d2[:, 0:128], in0=pp[:, 0:128], scalar=float(np.sqrt(W20)),
                    in1=sq[:, 0:128], op0=ALU.mult, op1=ALU.subtract)
                nc.vector.scalar_tensor_tensor(
                    d2[:, 128:256], in0=pp[:, 128:256],
                    scalar=float(np.sqrt(W10)),
                    in1=sq[:, 128:256], op0=ALU.mult, op1=ALU.subtract)
                rd2 = wrk.tile([128, 256], f32, tag="rd2")
                nc.vector.reciprocal_approx_fast(rd2[:], d2[:])
                ub = wrk.tile([128, 256], bf16, tag="ub")
                nc.scalar.activation(ub[:], rd2[:], ACT.Sqrt)
                # S' in bf16 for the q-columns of zp
                spb = wrk.tile([128, 256], bf16, tag="spb")
                nc.scalar.activation(spb[:], sp_[:], ACT.Copy)

                for (wi, R, msk) in ((0, R20, m20), (1, R10, m10)):
                    zp = psum([128, R + 128], "zp")
                    nc.tensor.matmul(
                        zp[:, 0:R], lhsT=ub[:, wi * 128:(wi + 1) * 128],
                        rhs=xTb[:, k * 128:k * 128 + R],
                        start=True, stop=True, skip_group_check=True)
                    nc.tensor.matmul(
                        zp[:, R:R + 128], lhsT=ub[:, wi * 128:(wi + 1) * 128],
                        rhs=spb[:, wi * 128:(wi + 1) * 128],
                        start=True, stop=True, skip_group_check=True)
                    V = wrk.tile([128, R + 128], f32, tag="V%d" % wi)
                    nc.scalar.activation(V[:], zp[:], ACT.Square)
                    scr = wrk.tile([128, R + 128], f32, tag="scr%d" % wi)
                    nc.gpsimd.tensor_mul(scr[:], V[:], msk[:])
                    dst = num20 if wi == 0 else num10
                    nc.vector.tensor_reduce(dst[:, k:k + 1], scr[:],
                                            axis=AX.X, op=ALU.add)

            if DO_ROLL:
                # phase locking count: num20 > thresh, masked valid
                cmp = sml.tile([128, 8], f32, tag="cmp")
                nc.vector.tensor_scalar(cmp[:], num20[:], THRESH20, None,
                                        ALU.is_gt)
                cmp2 = sml.tile([128, 8], f32, tag="cmp2")
                nc.gpsimd.tensor_mul(cmp2[:], cmp[:], v20[:])
                cnt = sml.tile([128, 1], f32, tag="cnt")
                nc.vector.tensor_reduce(cnt[:], cmp2[:], axis=AX.X, op=ALU.add)
                nc.vector.tensor_copy(slot(S_COUNT20), psum_scalar(cnt[:])[:])
                # raw hist/recent sums of the 10-window quadratic sums
                hv = sml.tile([128, 8], f32, tag="hv")
                nc.gpsimd.tensor_mul(hv[:], num10[:], h10[:])
                hs = sml.tile([128, 1], f32, tag="hs")
                nc.vector.tensor_reduce(hs[:], hv[:], axis=AX.X, op=ALU.add)
                nc.vector.tensor_copy(slot(S_HIST10), psum_scalar(hs[:])[:])
                rv = sml.tile([128, 8], f32, tag="rv")
                nc.gpsimd.tensor_mul(rv[:], num10[:], r10[:])
                rs = sml.tile([128, 1], f32, tag="rs")
                nc.vector.tensor_reduce(rs[:], rv[:], axis=AX.X, op=ALU.add)
                nc.vector.tensor_copy(slot(S_RECENT10), psum_scalar(rs[:])[:])

            # ================= cross-sectional std (herding) ===============
            if DO_CS:
                cs_s = per.tile([128, 8], f32, tag="cs_s")
                cs_q = per.tile([128, 8], f32, tag="cs_q")
                for b in range(8):
                    nc.vector.tensor_reduce(cs_s[:, b:b + 1], xcs[b][:],
                                            axis=AX.X, op=ALU.add)
                    nc.vector.tensor_reduce(cs_q[:, b:b + 1], xsqbs[b][:],
                                            axis=AX.X, op=ALU.add)
                cs_sq = sml.tile([128, 8], f32, tag="cs_sq")
                nc.scalar.activation(cs_sq[:], cs_s[:], ACT.Square)
                cs_var = sml.tile([128, 8], f32, tag="cs_var")
                nc.vector.scalar_tensor_tensor(
                    cs_var[:], in0=cs_sq[:], scalar=-1.0 / A, in1=cs_q[:],
                    op0=ALU.mult, op1=ALU.add)
                csstd = per.tile([128, 8], f32, tag="csstd")
                nc.scalar.activation(csstd[:], cs_var[:], ACT.Sqrt,
                                     scale=1.0 / (A - 1))
                csr = sml.tile([128, 1], f32, tag="csr")
                nc.vector.tensor_reduce(csr[:], csstd[:], axis=AX.X,
                                        op=ALU.add)
                nc.vector.tensor_copy(slot(S_CSSUM), psum_scalar(csr[:])[:])
                nc.vector.tensor_copy(slot(S_CSFIRST), csstd[0:1, 0:1])
                cslast_p = psum([1, 1], "sc")
                nc.tensor.matmul(cslast_p[:], lhsT=oh127[:], rhs=csstd[:, 7:8],
                                 start=True, stop=True, skip_group_check=True)
                nc.vector.tensor_copy(slot(S_CSLAST), cslast_p[:])

            # ================= cov postprocessing + eigenvalue =============
            if DO_COV:
                # diag, u = 1/sqrt(diag)
                dscr = wrk.tile([128, 128], f32, tag="dscr")
                nc.vector.tensor_mul(dscr[:], cov[:], ident[:])
                diag = per.tile([128, 1], f32, tag="diag")
                nc.vector.tensor_reduce(diag[:], dscr[:], axis=AX.X,
                                        op=ALU.add)
                dstd = per.tile([128, 1], f32, tag="dstd")
                nc.scalar.activation(dstd[:], diag[:], ACT.Sqrt)
                ucol = per.tile([128, 1], f32, tag="ucol")
                nc.vector.reciprocal(ucol[:], dstd[:])
                # trace(corr) = sum diag * u * u
                u2 = sml.tile([128, 1], f32, tag="u2")
                nc.vector.tensor_mul(u2[:], ucol[:], ucol[:])
                du2 = sml.tile([128, 1], f32, tag="du2")
                nc.vector.tensor_mul(du2[:], u2[:], diag[:])
                nc.vector.tensor_copy(slot(S_TRACE), psum_scalar(du2[:])[:])

                # u^T as a row for the quadratic forms
                uT_p = psum([1, 128], "sc")
                nc.tensor.transpose(uT_p[:], ucol[:], ident[:])
                uT = per.tile([1, 128], f32, tag="uT")
                nc.vector.tensor_copy(uT[:], uT_p[:])

                def quad_form(mat_sb, out_slot):
                    qr = psum([1, 128], "sc")
                    nc.tensor.matmul(qr[:], lhsT=ucol[:], rhs=mat_sb,
                                     start=True, stop=True,
                                     skip_group_check=True)
                    qscr = sml.tile([1, 128], f32, tag="qscr")
                    nc.vector.tensor_mul(qscr[:], qr[:], uT[:])
                    qacc = sml.tile([1, 1], f32, tag="qacc")
                    nc.vector.tensor_reduce(qacc[:], qscr[:], axis=AX.X,
                                            op=ALU.add)
                    nc.vector.tensor_copy(out_slot, qacc[:])

                quad_form(cov[:], slot(S_SUMCORR))
                acov = per.tile([128, 128], f32, tag="acov")
                nc.scalar.activation(acov[:], cov[:], ACT.Abs)
                quad_form(acov[:], slot(S_SUMABS))

                # corr = diag(u) cov diag(u) -> bf16
                brow = per.tile([128, 128], f32, tag="brow")
                nc.vector.tensor_scalar(brow[:], cov[:], ucol[:], None,
                                        ALU.mult)
                bt_p = psum([128, 128], "big")
                nc.tensor.transpose(bt_p[:], brow[:], ident[:])
                corr = per.tile([128, 128], bf16, tag="corr")
                nc.scalar.activation(corr[:], bt_p[:], ACT.Copy,
                                     scale=ucol[:])

            # ---- top eigenvalue: 9 bf16 squarings, one normalization ----
            if DO_COV and DO_EIG:
                def trace_of(p, out_slot):
                    escr = wrk.tile([128, 128], f32, tag="escr")
                    nc.vector.tensor_mul(escr[:], p[:], ident[:])
                    edg = sml.tile([128, 1], f32, tag="edg")
                    nc.vector.tensor_reduce(edg[:], escr[:], axis=AX.X,
                                            op=ALU.add)
                    trp = psum_scalar(edg[:])
                    tr_sb = sml.tile([1, 1], f32, tag="tr_sb")
                    nc.vector.tensor_copy(tr_sb[:], trp[:])
                    nc.vector.tensor_copy(out_slot, tr_sb[:])
                    return tr_sb

                M = corr
                for kk in range(9):
                    p = psum([128, 128], "big")
                    nc.tensor.matmul(p[:], lhsT=M[:], rhs=M[:],
                                     start=True, stop=True,
                                     skip_group_check=True)
                    Mn = wrk.tile([128, 128], bf16, tag="Mn")
                    if kk == 5:
                        t6 = trace_of(p, slot(S_T6))
                        rcp1 = sml.tile([1, 1], f32, tag="rcp1")
                        nc.vector.reciprocal_approx_fast(rcp1[:], t6[:])
                        bc = psum([128, 1], "sc")
                        nc.tensor.matmul(bc[:], lhsT=ones_row[:], rhs=rcp1[:],
                                         start=True, stop=True,
                                         skip_group_check=True)
                        bcc = sml.tile([128, 1], f32, tag="bcc")
                        nc.vector.tensor_copy(bcc[:], bc[:])
                        nc.scalar.activation(Mn[:], p[:], ACT.Copy,
                                             scale=bcc[:])
                    elif kk == 8:
                        trace_of(p, slot(S_T9))
                        continue
                    else:
                        nc.scalar.activation(Mn[:], p[:], ACT.Copy)
                    M = Mn

            # ================= position diversity =================
            pos_sb = per.tile([128, 1], f32, tag="pos_sb")
            nc.sync.dma_start(pos_sb[:], pos_in[:, :])
            if DO_POS:
                pa = per.tile([128, 1], f32, tag="pa")
                nc.scalar.activation(pa[:], pos_sb[:], ACT.Abs)
                nc.vector.tensor_copy(slot(S_PASUM), psum_scalar(pa[:])[:])
                paT_p = psum([1, 128], "sc")
                nc.tensor.transpose(paT_p[:], pa[:], ident[:])
                paT = sml.tile([1, 128], f32, tag="paT")
                nc.vector.tensor_copy(paT[:], paT_p[:])
                nc.vector.tensor_reduce(slot(S_PAMAX), paT[:], axis=AX.X,
                                        op=ALU.max)

            # ================= herding MLP =================
            if DO_MLP:
                w1a = cst.tile([128, 128], f32, tag="w1a")
                nc.sync.dma_start(w1a[:], w1a_in[:, :])
                w1b = cst.tile([128, 128], f32, tag="w1b")
                nc.sync.dma_start(w1b[:], w1b_in[:, :])
                b1 = cst.tile([128, 1], f32, tag="b1")
                nc.sync.dma_start(b1[:], b1_in[:, :])
                gam = cst.tile([128, 1], f32, tag="gam")
                nc.sync.dma_start(gam[:], gamma_in[:, :])
                bet = cst.tile([128, 1], f32, tag="bet")
                nc.sync.dma_start(bet[:], beta_in[:, :])
                w2 = cst.tile([128, 64], f32, tag="w2")
                nc.sync.dma_start(w2[:], w2_in[:, :])
                b2 = cst.tile([64, 1], f32, tag="b2")
                nc.sync.dma_start(b2[:], b2_in[:, :])
                w3 = cst.tile([64, 3], f32, tag="w3")
                nc.sync.dma_start(w3[:], w3_in[:, :])
                b3 = cst.tile([3, 1], f32, tag="b3")
                nc.sync.dma_start(b3[:], b3_in[:, :])
                xl = cst.tile([128, 1], f32, tag="xl")
                nc.sync.dma_start(xl[:], xlast_in[:, :])

                h1p = psum([128, 1], "sc")
                nc.tensor.matmul(h1p[:], lhsT=w1a[:], rhs=xl[:], start=True,
                                 stop=False, skip_group_check=True)
                nc.tensor.matmul(h1p[:], lhsT=w1b[:], rhs=pos_sb[:],
                                 start=False, stop=True,
                                 skip_group_check=True)
                h1 = sml.tile([128, 1], f32, tag="h1")
                nc.scalar.activation(h1[:], h1p[:], ACT.Relu, bias=b1[:])
                gk = sml.tile([128, 1], f32, tag="gk")
                nc.vector.tensor_scalar(gk[:], gam[:],
                                        float(1.0 / np.sqrt(1.0 + 1e-5)),
                                        None, ALU.mult)
                h1b = sml.tile([128, 1], f32, tag="h1b")
                nc.vector.tensor_scalar(h1b[:], h1[:], gk[:], bet[:],
                                        ALU.mult, ALU.add)
                h2p = psum([64, 1], "sc")
                nc.tensor.matmul(h2p[:], lhsT=w2[:], rhs=h1b[:], start=True,
                                 stop=True, skip_group_check=True)
                h2 = sml.tile([64, 1], f32, tag="h2")
                nc.scalar.activation(h2[:], h2p[:], ACT.Relu, bias=b2[:])
                lg = psum([3, 1], "sc")
                nc.tensor.matmul(lg[:], lhsT=w3[:], rhs=h2[:], start=True,
                                 stop=True, skip_group_check=True)
                exps = sml.tile([3, 1], f32, tag="exps")
                nc.scalar.activation(exps[:], lg[:], ACT.Exp, bias=b3[:])
                esum = psum_scalar(exps[:], p=3)
                esum_sb = sml.tile([1, 1], f32, tag="esum_sb")
                nc.vector.tensor_copy(esum_sb[:], esum[:])
                erec = sml.tile([1, 1], f32, tag="erec")
                nc.vector.reciprocal(erec[:], esum_sb[:])
                e2p = psum([1, 1], "sc")
                nc.tensor.matmul(e2p[:], lhsT=oh2[:], rhs=exps[:], start=True,
                                 stop=True, skip_group_check=True)
                e2_sb = sml.tile([1, 1], f32, tag="e2_sb")
                nc.vector.tensor_copy(e2_sb[:], e2p[:])
                nc.vector.tensor_mul(slot(S_SEV), e2_sb[:], erec[:])

            # ================= write out =================
            nc.sync.dma_start(out_d[:, :], out_sb[:])

    nc.compile()
    return nc


def _prep_in_maps(inputs):
    x = np.ascontiguousarray(np.asarray(inputs["returns_sequence"],
                                        dtype=np.float32))
    pos = np.asarray(inputs["positions"], np.float32).reshape(128, 1)
    m20, m10 = _build_masks()
    b0, b1 = _build_bands()
    ident = np.eye(128, dtype=np.float32)
    w1 = np.asarray(inputs["w1"], np.float32)
    common = {
        "bands0": b0, "bands1": b1,
        "mask20": m20, "mask10": m10, "ident": ident,
        "w1a": np.ascontiguousarray(w1[0:128]),
        "w1b": np.ascontiguousarray(w1[128:256]),
        "b1": np.asarray(inputs["b1"], np.float32).reshape(128, 1),
        "gamma": np.asarray(inputs["gamma"], np.float32).reshape(128, 1),
        "beta": np.asarray(inputs["beta"], np.float32).reshape(128, 1),
        "w2": np.asarray(inputs["w2"], np.float32),
        "b2": np.asarray(inputs["b2"], np.float32).reshape(64, 1),
        "w3": np.asarray(inputs["w3"], np.float32),
        "b3": np.asarray(inputs["b3"], np.float32).reshape(3, 1),
        "positions": pos,
        "xlast": np.ascontiguousarray(x[-1].reshape(128, 1)),
        "onehot127": np.eye(128, dtype=np.float32)[:, 127:128].copy(),
        "onehot2": np.array([[0.0], [0.0], [1.0]], np.float32),
    }
    in_maps = []
    for c in range(NC_N):
        rows = (c * CHUNK + np.arange(XROWS)) % T
        v20, h10, r10 = _core_masks(c)
        m = dict(common)
        m["x_chunk"] = np.ascontiguousarray(x[rows])
        m["valid20"] = v20
        m["hist10"] = h10
        m["recent10"] = r10
        in_maps.append(m)
    return in_maps


def _combine(per_core):
    count20 = sum(float(per_core[c][0, S_COUNT20]) for c in range(NC_N))
    hist_raw = sum(float(per_core[c][0, S_HIST10]) for c in range(NC_N))
    rec_raw = sum(float(per_core[c][0, S_RECENT10]) for c in range(NC_N))
    cs_sum = sum(float(per_core[c][0, S_CSSUM]) for c in range(NC_N))
    ssq_sum = sum(float(per_core[c][0, S_SSQ]) for c in range(NC_N))
    cs_first = float(per_core[0][0, S_CSFIRST])
    cs_last = float(per_core[NC_N - 1][0, S_CSLAST])
    r0 = per_core[0][0]
    sum_corr = float(r0[S_SUMCORR])
    sum_abs = float(r0[S_SUMABS])
    trace_c = float(r0[S_TRACE])
    pa_sum = float(r0[S_PASUM])
    pa_max = float(r0[S_PAMAX])
    severity = float(r0[S_SEV])
    T6, T9 = float(r0[S_T6]), float(r0[S_T9])

    phase_locking = count20 / N20
    nh = N10 - 5
    hist = (hist_raw - nh * A) * INV_OD / nh
    recent = (rec_raw - 5 * A) * INV_OD / 5.0
    surge = 0.0
    if hist > 0:
        surge = min(max((recent - hist) / hist, 0.0), 1.0)
    avg_disp = cs_sum / T
    trend = -(cs_last - cs_first) / (T - 1)
    herding_index = min(max(trend / (avg_disp + 1e-6) + 0.5, 0.0), 1.0)
    avg_corr = (sum_corr - trace_c) / (A * (A - 1))
    lam = np.exp((8.0 * np.log(T6) + np.log(T9)) / 512.0)
    sync_risk = min(1.0, (lam / A) * avg_corr)
    return_div = 1.0 - sum_abs / (A * A)
    pos_div = 1.0 - pa_max / pa_sum
    div_loss = 1.0 - np.sqrt(return_div * pos_div)
    avg_conc = (A * A / 2.0 + ssq_sum / (2.0 * T) - A) / (A * (A - 1))
    phase_coupling = min(max((avg_conc - 0.5) * 2.0, 0.0), 1.0)
    collective = (herding_index + sync_risk + div_loss) / 3.0
    return np.array([herding_index, severity, sync_risk, phase_locking,
                     div_loss, surge, phase_coupling, collective],
                    dtype=np.float32)


def _ensure_ntff_hook():
    """Install the axon NTFF profile hook if the image lacks antenv.axon_hooks."""
    import sys
    import types
    try:
        import antenv.axon_hooks  # noqa: F401
        return True
    except ImportError:
        pass
    try:
        import antenv
        from trn_agent_boot.trn_boot import _ntff_profile_via_ctypes
        mod = types.ModuleType("antenv.axon_hooks")
        state = {}
        mod.set_axon_ntff_profile_hook = lambda h: state.update(h=h)
        mod.get_axon_ntff_profile_hook = lambda: state.get("h")
        sys.modules["antenv.axon_hooks"] = mod
        antenv.axon_hooks = mod
        hook = _ntff_profile_via_ctypes("/opt/axon/libaxon_pjrt.so")
        mod.set_axon_ntff_profile_hook(hook)
        return hook is not None
    except Exception:
        return False


def _run(inputs, trace=False):
    from concourse.bass_utils import run_bass_kernel_spmd
    if trace:
        trace = _ensure_ntff_hook()
    if "nc" not in _PLAN:
        _PLAN["nc"] = _build_program()
    nc = _PLAN["nc"]
    in_maps = _prep_in_maps(inputs)
    res = run_bass_kernel_spmd(nc, in_maps, core_ids=list(range(NC_N)),
                               trace=trace)
    per_core = [res.results[c]["out_vec"] for c in range(NC_N)]
    return _combine(per_core), res


def kernel(**inputs) -> np.ndarray:
    out, _ = _run(inputs, trace=False)
    return out


# revision 7
# speedup vs baseline: 1.7489x; 1.7321x over previous
"""Trainium2 Bass kernel for nn_EmergentRiskMetrics.

Contract: kernel(**inputs) takes the FULL unsharded inputs (as produced by
setup_inputs()) and returns the FULL output (shape [8], float32).

Sharding: data-parallel over the time axis. Each of the 8 cores owns 1024
contiguous window starts (plus a 128-row halo) for the two rolling-window
correlation scans; the sign-concordance partial sum and cross-sectional
stds are computed on the owning core and combined as scalars on the host.
The full-T covariance (needed on-device for the eigenvalue iteration) is
replicated: every core re-computes X^T X from bf16 tiles of the full
sequence (cheap 128^3 bf16 matmuls + ~2 MB of DMA, overlapped with the
rolling phase). An earlier AllReduce-based variant was measured at ~66 us
of pure collective latency for 64 KB on this runtime — replication is far
cheaper.

Windowed sums are computed with banded-matrix matmuls on the tensor engine
(bands pre-scaled by 1/sqrt(w) so the mean-correction q^2 term folds into
the existing V-mask), u = 1/std via reciprocal_approx_fast + Sqrt, and the
whole rolling path runs in bf16 (validated: all rolling-derived outputs
have orders-of-magnitude margin against bf16 noise). The host pre-casts x
to bf16 (and pre-transposes the per-core chunk), which halves DMA bytes,
removes all on-device transposes/casts, and makes d2 = P - S^2/w >= 0
exact by Cauchy-Schwarz.

Top eigenvalue: corr is squared 9 times in bf16 (fp32 PSUM accumulate);
traces at step 6 (normalization) and step 9 give lam = (T9*T6^8)^(1/512)
on the host.

Device outputs are per-core partial scalars; the host only gathers them
(sums partial sums, applies the final scalar clips/divides) to assemble
the 8 outputs.
"""

import numpy as np

T = 8192
A = 128
W20 = 20
W10 = 10
NC_N = 8
CHUNK = 1024            # window starts per core
XROWS = 1152            # rows of per-core chunk (9 x 128, incl. halo)
NBLK = XROWS // 128     # 9
R20 = 128 + W20 - 1     # 147
R10 = 128 + W10 - 1     # 137
N20 = T - W20           # 8172 rolling-20 windows
N10 = T - W10           # 8182 rolling-10 windows
OUT_SLOTS = 24
INV_OD = 1.0 / (A * (A - 1))
# rolling20 > 0.7 in corr units == raw quadratic sum > this
THRESH20 = 0.7 * (A * (A - 1)) + A

S_COUNT20, S_HIST10, S_RECENT10, S_CSSUM, S_CSFIRST, S_CSLAST, \
    S_SUMCORR, S_SUMABS, S_TRACE, S_PASUM, S_PAMAX, S_SEV, S_SSQ, \
    S_T6, S_T9 = range(15)

_PLAN = {}


def _build_masks():
    # V-masks: +1 over the window band, -1 on the q^2 column
    m20 = np.zeros((128, R20 + 128), np.float32)
    m10 = np.zeros((128, R10 + 128), np.float32)
    for j in range(128):
        m20[j, j:j + W20] = 1.0
        m20[j, R20 + j] = -1.0
        m10[j, j:j + W10] = 1.0
        m10[j, R10 + j] = -1.0
    return m20, m10


def _build_bands():
    # bands0/1 [128 t, 256]: cols 0:128 window-20 (scaled 1/sqrt20),
    # cols 128:256 window-10 (scaled 1/sqrt10). S' = B0^T x_k + B1^T x_{k+1}
    b0 = np.zeros((128, 256), np.float32)
    b1 = np.zeros((128, 256), np.float32)
    s20 = 1.0 / np.sqrt(W20)
    s10 = 1.0 / np.sqrt(W10)
    for j in range(128):
        lo20, hi20 = j, j + W20
        lo10, hi10 = j, j + W10
        b0[max(0, lo20):min(128, hi20), j] = s20
        if hi20 > 128:
            b1[0:hi20 - 128, j] = s20
        b0[max(0, lo10):min(128, hi10), 128 + j] = s10
        if hi10 > 128:
            b1[0:hi10 - 128, 128 + j] = s10
    return b0, b1


def _core_masks(c):
    g = c * CHUNK + np.arange(CHUNK)
    valid20 = (g < N20).astype(np.float32)
    hist10 = (g < N10 - 5).astype(np.float32)
    recent10 = ((g >= N10 - 5) & (g < N10)).astype(np.float32)
    # device layout [128 partitions (j in chunk), 8 chunk-columns]
    return (np.ascontiguousarray(valid20.reshape(8, 128).T),
            np.ascontiguousarray(hist10.reshape(8, 128).T),
            np.ascontiguousarray(recent10.reshape(8, 128).T))


def _build_program():
    import os
    import concourse.bacc as bacc
    import concourse.tile as tile
    from concourse import mybir

    kbits = int(os.environ.get("KBITS", "63"))
    bigdma = int(os.environ.get("BIGDMA", "1"))
    DO_ROLL = kbits & 1
    DO_CS = kbits & 2
    DO_COV = kbits & 4
    DO_EIG = kbits & 8
    DO_POS = kbits & 16
    DO_MLP = kbits & 32

    f32 = mybir.dt.float32
    bf16 = mybir.dt.bfloat16
    ALU = mybir.AluOpType
    ACT = mybir.ActivationFunctionType
    AX = mybir.AxisListType

    nc = bacc.Bacc("TRN2", target_bir_lowering=False, debug=False,
                   num_devices=NC_N)

    def din(name, shape, dt=f32):
        return nc.dram_tensor(name, shape, dt, kind="ExternalInput").ap()

    x_full_b = din("x_full_b", [T, A], bf16)
    xchunk_b = din("xchunk_b", [XROWS, A], bf16)
    xT_b_in = din("xT_b", [128, XROWS], bf16)
    bands0_in = din("bands0", [128, 256], bf16)
    bands1_in = din("bands1", [128, 256], bf16)
    mask20 = din("mask20", [128, R20 + 128])
    mask10 = din("mask10", [128, R10 + 128])
    valid20 = din("valid20", [128, 8])
    hist10 = din("hist10", [128, 8])
    recent10 = din("recent10", [128, 8])
    ident_in = din("ident", [128, 128])
    w1a_in = din("w1a", [128, 128])
    w1b_in = din("w1b", [128, 128])
    b1_in = din("b1", [128, 1])
    gamma_in = din("gamma", [128, 1])
    beta_in = din("beta", [128, 1])
    w2_in = din("w2", [128, 64])
    b2_in = din("b2", [64, 1])
    w3_in = din("w3", [64, 3])
    b3_in = din("b3", [3, 1])
    pos_in = din("positions", [128, 1])
    xlast_in = din("xlast", [128, 1])
    oh127_in = din("onehot127", [128, 1])
    oh2_in = din("onehot2", [3, 1])
    out_d = nc.dram_tensor("out_vec", [1, OUT_SLOTS], f32,
                           kind="ExternalOutput").ap()

    with tile.TileContext(nc) as tc:
        with tc.tile_pool(name="const", bufs=1) as cst, \
             tc.tile_pool(name="persist", bufs=1) as per, \
             tc.tile_pool(name="sgs", bufs=3) as sgs, \
             tc.tile_pool(name="wrk", bufs=3) as wrk, \
             tc.tile_pool(name="small", bufs=6) as sml, \
             tc.tile_pool(name="ps", bufs=1, space="PSUM") as ps:

            psum_bufs = {"covq": 1, "band": 2, "zp": 2, "big": 1, "sc": 2}

            def psum(shape, tag):
                return ps.tile(shape, f32, tag=tag, name=tag,
                               bufs=psum_bufs[tag])

            # ---- constants (sync queue) ----
            ident = cst.tile([128, 128], f32, tag="ident")
            nc.sync.dma_start(ident[:], ident_in[:, :])
            b0b = cst.tile([128, 256], bf16, tag="b0b")
            nc.sync.dma_start(b0b[:], bands0_in[:, :])
            b1b = cst.tile([128, 256], bf16, tag="b1b")
            nc.sync.dma_start(b1b[:], bands1_in[:, :])
            m20 = cst.tile([128, R20 + 128], f32, tag="m20")
            nc.sync.dma_start(m20[:], mask20[:, :])
            m10 = cst.tile([128, R10 + 128], f32, tag="m10")
            nc.sync.dma_start(m10[:], mask10[:, :])
            v20 = cst.tile([128, 8], f32, tag="v20")
            nc.sync.dma_start(v20[:], valid20[:, :])
            h10 = cst.tile([128, 8], f32, tag="h10")
            nc.sync.dma_start(h10[:], hist10[:, :])
            r10 = cst.tile([128, 8], f32, tag="r10")
            nc.sync.dma_start(r10[:], recent10[:, :])
            ones = cst.tile([128, 1], f32, tag="ones")
            nc.vector.memset(ones[:], 1.0)
            ones_row = cst.tile([1, 128], f32, tag="ones_row")
            nc.vector.memset(ones_row[:], 1.0)
            oh127 = cst.tile([128, 1], f32, tag="oh127")
            nc.sync.dma_start(oh127[:], oh127_in[:, :])
            oh2 = cst.tile([3, 1], f32, tag="oh2")
            nc.sync.dma_start(oh2[:], oh2_in[:, :])

            # MLP consts early so the MLP can run during startup
            w1a = cst.tile([128, 128], f32, tag="w1a")
            nc.sync.dma_start(w1a[:], w1a_in[:, :])
            w1b = cst.tile([128, 128], f32, tag="w1b")
            nc.sync.dma_start(w1b[:], w1b_in[:, :])
            b1 = cst.tile([128, 1], f32, tag="b1")
            nc.sync.dma_start(b1[:], b1_in[:, :])
            gam = cst.tile([128, 1], f32, tag="gam")
            nc.sync.dma_start(gam[:], gamma_in[:, :])
            bet = cst.tile([128, 1], f32, tag="bet")
            nc.sync.dma_start(bet[:], beta_in[:, :])
            w2 = cst.tile([128, 64], f32, tag="w2")
            nc.sync.dma_start(w2[:], w2_in[:, :])
            b2 = cst.tile([64, 1], f32, tag="b2")
            nc.sync.dma_start(b2[:], b2_in[:, :])
            w3 = cst.tile([64, 3], f32, tag="w3")
            nc.sync.dma_start(w3[:], w3_in[:, :])
            b3 = cst.tile([3, 1], f32, tag="b3")
            nc.sync.dma_start(b3[:], b3_in[:, :])
            xl = cst.tile([128, 1], f32, tag="xl")
            nc.sync.dma_start(xl[:], xlast_in[:, :])
            pos_sb = per.tile([128, 1], f32, tag="pos_sb")
            nc.sync.dma_start(pos_sb[:], pos_in[:, :])

            out_sb = per.tile([1, OUT_SLOTS], f32, tag="out_sb")
            nc.vector.memset(out_sb[:], 0.0)

            def slot(i):
                return out_sb[:, i:i + 1]

            def psum_scalar(vec_sb, p=128):
                o = psum([1, 1], "sc")
                lhs = ones[0:p, :] if p != 128 else ones[:]
                nc.tensor.matmul(o[:], lhsT=lhs, rhs=vec_sb,
                                 start=True, stop=True, skip_group_check=True)
                return o

            # ---- bulk loads: xT (one DMA), chunk tiles, full-x tiles ----
            xTb = per.tile([128, XROWS], bf16, tag="xTb")
            nc.sync.dma_start(xTb[:], xT_b_in[:, :])

            xcbs = []
            for j in range(NBLK):
                xcb = per.tile([128, 128], bf16, tag="xcb%d" % j)
                nc.sync.dma_start(xcb[:], xchunk_b[j * 128:(j + 1) * 128, :])
                xcbs.append(xcb)

            # full-x tiles for the replicated covariance, split across the
            # two HWDGE queues (sync + scalar)
            xfbs = []
            if DO_COV:
                if bigdma:
                    for i in range(16):
                        xf = per.tile([128, 512], bf16, tag="xf%d" % i)
                        src = x_full_b[i * 512:(i + 1) * 512, :].rearrange(
                            "(j p) c -> p j c", p=128)
                        dst = xf[:].rearrange("p (j c) -> p j c", j=4)
                        eng = nc.sync if i % 2 == 0 else nc.scalar
                        eng.dma_start(dst, src)
                        xfbs.extend(xf[:, j * 128:(j + 1) * 128]
                                    for j in range(4))
                else:
                    for i in range(64):
                        xf = per.tile([128, 128], bf16, tag="xs%d" % i)
                        eng = nc.sync if i % 2 == 0 else nc.scalar
                        eng.dma_start(xf[:], x_full_b[i * 128:(i + 1) * 128, :])
                        xfbs.append(xf[:])

            # ---- per-tile preps: squares (gpsimd) + signs (scalar) ----
            xsqbs = []
            for j in range(NBLK):
                xsqb = per.tile([128, 128], bf16, tag="xsqb%d" % j)
                nc.gpsimd.tensor_mul(xsqb[:], xcbs[j][:], xcbs[j][:])
                xsqbs.append(xsqb)

            # ---- sharded sign concordance ----
            mq = psum([128, 128], "big")
            for i in range(8):
                sg = sgs.tile([128, 128], bf16, tag="sg")
                nc.scalar.activation(sg[:], xcbs[i][:], ACT.Sign)
                nc.tensor.matmul(mq[:], lhsT=sg[:], rhs=sg[:],
                                 start=(i == 0), stop=(i == 7),
                                 skip_group_check=True)
            mr = sml.tile([128, 1], f32, tag="mr")
            nc.vector.tensor_reduce(mr[:], mq[:], axis=AX.X, op=ALU.add)
            nc.vector.tensor_copy(slot(S_SSQ), psum_scalar(mr[:])[:])

            # ---- cross-sectional sums (independent; fills startup) ----
            if DO_CS:
                cs_s = per.tile([128, 8], f32, tag="cs_s")
                cs_q = per.tile([128, 8], f32, tag="cs_q")
                for b in range(8):
                    nc.vector.tensor_reduce(cs_s[:, b:b + 1], xcbs[b][:],
                                            axis=AX.X, op=ALU.add)
                    nc.vector.tensor_reduce(cs_q[:, b:b + 1], xsqbs[b][:],
                                            axis=AX.X, op=ALU.add)

            # ================= rolling windows + cov, interleaved ==========
            covq = psum([128, 128], "covq")
            num20 = per.tile([128, 8], f32, tag="num20")
            num10 = per.tile([128, 8], f32, tag="num10")
            for k in range(8):
                if DO_ROLL:
                    sp_ = psum([128, 256], "band")
                    nc.tensor.matmul(sp_[:], lhsT=xcbs[k][:], rhs=b0b[:],
                                     start=True, stop=False,
                                     skip_group_check=True)
                    nc.tensor.matmul(sp_[:], lhsT=xcbs[k + 1][:], rhs=b1b[:],
                                     start=False, stop=True,
                                     skip_group_check=True)
                    pp = psum([128, 256], "band")
                    nc.tensor.matmul(pp[:], lhsT=xsqbs[k][:], rhs=b0b[:],
                                     start=True, stop=False,
                                     skip_group_check=True)
                    nc.tensor.matmul(pp[:], lhsT=xsqbs[k + 1][:], rhs=b1b[:],
                                     start=False, stop=True,
                                     skip_group_check=True)
                    # d2 = P - S^2/w = pp*sqrt(w) - S'^2 (>=0 exactly)
                    sq = wrk.tile([128, 256], f32, tag="sq")
                    nc.scalar.activation(sq[:], sp_[:], ACT.Square)
                    d2 = wrk.tile([128, 256], f32, tag="d2")
                    nc.vector.scalar_tensor_tensor(
                        d2[:, 0:128], in0=pp[:, 0:128],
                        scalar=float(np.sqrt(W20)),
                        in1=sq[:, 0:128], op0=ALU.mult, op1=ALU.subtract)
                    nc.vector.scalar_tensor_tensor(
                        d2[:, 128:256], in0=pp[:, 128:256],
                        scalar=float(np.sqrt(W10)),
                        in1=sq[:, 128:256], op0=ALU.mult, op1=ALU.subtract)
                    rd2 = wrk.tile([128, 256], f32, tag="rd2")
                    nc.vector.reciprocal_approx_fast(rd2[:], d2[:])
                    ub = wrk.tile([128, 256], bf16, tag="ub")
                    nc.scalar.activation(ub[:], rd2[:], ACT.Sqrt)
                    spb = wrk.tile([128, 256], bf16, tag="spb")
                    nc.vector.tensor_copy(spb[:], sp_[:])

                    for (wi, R, msk) in ((0, R20, m20), (1, R10, m10)):
                        zp = psum([128, R + 128], "zp")
                        nc.tensor.matmul(
                            zp[:, 0:R], lhsT=ub[:, wi * 128:(wi + 1) * 128],
                            rhs=xTb[:, k * 128:k * 128 + R],
                            start=True, stop=True, skip_group_check=True)
                        nc.tensor.matmul(
                            zp[:, R:R + 128],
                            lhsT=ub[:, wi * 128:(wi + 1) * 128],
                            rhs=spb[:, wi * 128:(wi + 1) * 128],
                            start=True, stop=True, skip_group_check=True)
                        V = wrk.tile([128, R + 128], f32, tag="V%d" % wi)
                        nc.scalar.activation(V[:], zp[:], ACT.Square)
                        scr = wrk.tile([128, R + 128], f32, tag="scr%d" % wi)
                        nc.gpsimd.tensor_mul(scr[:], V[:], msk[:])
                        dst = num20 if wi == 0 else num10
                        nc.vector.tensor_reduce(dst[:, k:k + 1], scr[:],
                                                axis=AX.X, op=ALU.add)
                # 8 of the 64 replicated cov matmuls per rolling chunk
                for i in range(8 if DO_COV else 0):
                    t_ = xfbs[k * 8 + i]
                    nc.tensor.matmul(covq[:], lhsT=t_, rhs=t_,
                                     start=(k == 0 and i == 0),
                                     stop=(k == 7 and i == 7),
                                     skip_group_check=True)

            if DO_ROLL:
                # phase locking count: num20 > thresh, masked valid
                cmp = sml.tile([128, 8], f32, tag="cmp")
                nc.vector.tensor_scalar(cmp[:], num20[:], THRESH20, None,
                                        ALU.is_gt)
                cmp2 = sml.tile([128, 8], f32, tag="cmp2")
                nc.gpsimd.tensor_mul(cmp2[:], cmp[:], v20[:])
                cnt = sml.tile([128, 1], f32, tag="cnt")
                nc.vector.tensor_reduce(cnt[:], cmp2[:], axis=AX.X,
                                        op=ALU.add)
                nc.vector.tensor_copy(slot(S_COUNT20), psum_scalar(cnt[:])[:])
                hv = sml.tile([128, 8], f32, tag="hv")
                nc.gpsimd.tensor_mul(hv[:], num10[:], h10[:])
                hs = sml.tile([128, 1], f32, tag="hs")
                nc.vector.tensor_reduce(hs[:], hv[:], axis=AX.X, op=ALU.add)
                nc.vector.tensor_copy(slot(S_HIST10), psum_scalar(hs[:])[:])
                rv = sml.tile([128, 8], f32, tag="rv")
                nc.gpsimd.tensor_mul(rv[:], num10[:], r10[:])
                rs = sml.tile([128, 1], f32, tag="rs")
                nc.vector.tensor_reduce(rs[:], rv[:], axis=AX.X, op=ALU.add)
                nc.vector.tensor_copy(slot(S_RECENT10), psum_scalar(rs[:])[:])

            # ---- cross-sectional std finish ----
            if DO_CS:
                cs_sq = sml.tile([128, 8], f32, tag="cs_sq")
                nc.scalar.activation(cs_sq[:], cs_s[:], ACT.Square)
                cs_var = sml.tile([128, 8], f32, tag="cs_var")
                nc.vector.scalar_tensor_tensor(
                    cs_var[:], in0=cs_sq[:], scalar=-1.0 / A, in1=cs_q[:],
                    op0=ALU.mult, op1=ALU.add)
                csstd = per.tile([128, 8], f32, tag="csstd")
                nc.scalar.activation(csstd[:], cs_var[:], ACT.Sqrt,
                                     scale=1.0 / (A - 1))
                csr = sml.tile([128, 1], f32, tag="csr")
                nc.vector.tensor_reduce(csr[:], csstd[:], axis=AX.X,
                                        op=ALU.add)
                nc.vector.tensor_copy(slot(S_CSSUM), psum_scalar(csr[:])[:])
                nc.vector.tensor_copy(slot(S_CSFIRST), csstd[0:1, 0:1])
                cslast_p = psum([1, 1], "sc")
                nc.tensor.matmul(cslast_p[:], lhsT=oh127[:], rhs=csstd[:, 7:8],
                                 start=True, stop=True, skip_group_check=True)
                nc.vector.tensor_copy(slot(S_CSLAST), cslast_p[:])

            # ================= position diversity =================
            if DO_POS:
                pa = per.tile([128, 1], f32, tag="pa")
                nc.scalar.activation(pa[:], pos_sb[:], ACT.Abs)
                nc.vector.tensor_copy(slot(S_PASUM), psum_scalar(pa[:])[:])
                paT_p = psum([1, 128], "sc")
                nc.tensor.transpose(paT_p[:], pa[:], ident[:])
                paT = sml.tile([1, 128], f32, tag="paT")
                nc.vector.tensor_copy(paT[:], paT_p[:])
                nc.vector.tensor_reduce(slot(S_PAMAX), paT[:], axis=AX.X,
                                        op=ALU.max)

            # ================= herding MLP =================
            if DO_MLP:
                h1p = psum([128, 1], "sc")
                nc.tensor.matmul(h1p[:], lhsT=w1a[:], rhs=xl[:], start=True,
                                 stop=False, skip_group_check=True)
                nc.tensor.matmul(h1p[:], lhsT=w1b[:], rhs=pos_sb[:],
                                 start=False, stop=True,
                                 skip_group_check=True)
                h1 = sml.tile([128, 1], f32, tag="h1")
                nc.scalar.activation(h1[:], h1p[:], ACT.Relu, bias=b1[:])
                gk = sml.tile([128, 1], f32, tag="gk")
                nc.vector.tensor_scalar(gk[:], gam[:],
                                        float(1.0 / np.sqrt(1.0 + 1e-5)),
                                        None, ALU.mult)
                h1b = sml.tile([128, 1], f32, tag="h1b")
                nc.vector.tensor_scalar(h1b[:], h1[:], gk[:], bet[:],
                                        ALU.mult, ALU.add)
                h2p = psum([64, 1], "sc")
                nc.tensor.matmul(h2p[:], lhsT=w2[:], rhs=h1b[:], start=True,
                                 stop=True, skip_group_check=True)
                h2 = sml.tile([64, 1], f32, tag="h2")
                nc.scalar.activation(h2[:], h2p[:], ACT.Relu, bias=b2[:])
                lg = psum([3, 1], "sc")
                nc.tensor.matmul(lg[:], lhsT=w3[:], rhs=h2[:], start=True,
                                 stop=True, skip_group_check=True)
                exps = sml.tile([3, 1], f32, tag="exps")
                nc.scalar.activation(exps[:], lg[:], ACT.Exp, bias=b3[:])
                esum = psum_scalar(exps[:], p=3)
                esum_sb = sml.tile([1, 1], f32, tag="esum_sb")
                nc.vector.tensor_copy(esum_sb[:], esum[:])
                erec = sml.tile([1, 1], f32, tag="erec")
                nc.vector.reciprocal(erec[:], esum_sb[:])
                e2p = psum([1, 1], "sc")
                nc.tensor.matmul(e2p[:], lhsT=oh2[:], rhs=exps[:], start=True,
                                 stop=True, skip_group_check=True)
                e2_sb = sml.tile([1, 1], f32, tag="e2_sb")
                nc.vector.tensor_copy(e2_sb[:], e2p[:])
                nc.vector.tensor_mul(slot(S_SEV), e2_sb[:], erec[:])

            # ================= cov postprocessing + eigenvalue =============
            if DO_COV:
                cov = per.tile([128, 128], f32, tag="cov")
                nc.scalar.activation(cov[:], covq[:], ACT.Copy)
                dscr = wrk.tile([128, 128], f32, tag="dscr")
                nc.vector.tensor_mul(dscr[:], cov[:], ident[:])
                diag = per.tile([128, 1], f32, tag="diag")
                nc.vector.tensor_reduce(diag[:], dscr[:], axis=AX.X,
                                        op=ALU.add)
                dstd = per.tile([128, 1], f32, tag="dstd")
                nc.scalar.activation(dstd[:], diag[:], ACT.Sqrt)
                ucol = per.tile([128, 1], f32, tag="ucol")
                nc.vector.reciprocal(ucol[:], dstd[:])
                u2 = sml.tile([128, 1], f32, tag="u2")
                nc.vector.tensor_mul(u2[:], ucol[:], ucol[:])
                du2 = sml.tile([128, 1], f32, tag="du2")
                nc.vector.tensor_mul(du2[:], u2[:], diag[:])
                nc.vector.tensor_copy(slot(S_TRACE), psum_scalar(du2[:])[:])

                uT_p = psum([1, 128], "sc")
                nc.tensor.transpose(uT_p[:], ucol[:], ident[:])
                uT = per.tile([1, 128], f32, tag="uT")
                nc.vector.tensor_copy(uT[:], uT_p[:])

                def quad_form(mat_sb, out_slot):
                    qr = psum([1, 128], "sc")
                    nc.tensor.matmul(qr[:], lhsT=ucol[:], rhs=mat_sb,
                                     start=True, stop=True,
                                     skip_group_check=True)
                    qscr = sml.tile([1, 128], f32, tag="qscr")
                    nc.vector.tensor_mul(qscr[:], qr[:], uT[:])
                    qacc = sml.tile([1, 1], f32, tag="qacc")
                    nc.vector.tensor_reduce(qacc[:], qscr[:], axis=AX.X,
                                            op=ALU.add)
                    nc.vector.tensor_copy(out_slot, qacc[:])

                quad_form(cov[:], slot(S_SUMCORR))
                acov = per.tile([128, 128], f32, tag="acov")
                nc.scalar.activation(acov[:], cov[:], ACT.Abs)
                quad_form(acov[:], slot(S_SUMABS))

                # corr = diag(u) cov diag(u) -> bf16
                brow = per.tile([128, 128], f32, tag="brow")
                nc.vector.tensor_scalar(brow[:], cov[:], ucol[:], None,
                                        ALU.mult)
                bt_p = psum([128, 128], "big")
                nc.tensor.transpose(bt_p[:], brow[:], ident[:])
                corr = per.tile([128, 128], bf16, tag="corr")
                nc.scalar.activation(corr[:], bt_p[:], ACT.Copy,
                                     scale=ucol[:])

            if DO_COV and DO_EIG:
                def trace_of(p, out_slot):
                    escr = wrk.tile([128, 128], f32, tag="escr")
                    nc.vector.tensor_mul(escr[:], p[:], ident[:])
                    edg = sml.tile([128, 1], f32, tag="edg")
                    nc.vector.tensor_reduce(edg[:], escr[:], axis=AX.X,
                                            op=ALU.add)
                    trp = psum_scalar(edg[:])
                    tr_sb = sml.tile([1, 1], f32, tag="tr_sb")
                    nc.vector.tensor_copy(tr_sb[:], trp[:])
                    nc.vector.tensor_copy(out_slot, tr_sb[:])
                    return tr_sb

                M = corr
                for kk in range(9):
                    p = psum([128, 128], "big")
                    nc.tensor.matmul(p[:], lhsT=M[:], rhs=M[:],
                                     start=True, stop=True,
                                     skip_group_check=True)
                    Mn = wrk.tile([128, 128], bf16, tag="Mn")
                    if kk == 5:
                        t6 = trace_of(p, slot(S_T6))
                        rcp1 = sml.tile([1, 1], f32, tag="rcp1")
                        nc.vector.reciprocal_approx_fast(rcp1[:], t6[:])
                        bc = psum([128, 1], "sc")
                        nc.tensor.matmul(bc[:], lhsT=ones_row[:], rhs=rcp1[:],
                                         start=True, stop=True,
                                         skip_group_check=True)
                        bcc = sml.tile([128, 1], f32, tag="bcc")
                        nc.vector.tensor_copy(bcc[:], bc[:])
                        nc.scalar.activation(Mn[:], p[:], ACT.Copy,
                                             scale=bcc[:])
                    elif kk == 8:
                        trace_of(p, slot(S_T9))
                        continue
                    else:
                        nc.scalar.activation(Mn[:], p[:], ACT.Copy)
                    M = Mn

            # ================= write out =================
            nc.sync.dma_start(out_d[:, :], out_sb[:])

    nc.compile()
    return nc


def _prep_in_maps(inputs):
    import ml_dtypes
    bfloat16 = ml_dtypes.bfloat16
    x = np.ascontiguousarray(np.asarray(inputs["returns_sequence"],
                                        dtype=np.float32))
    xb = x.astype(bfloat16)
    pos = np.asarray(inputs["positions"], np.float32).reshape(128, 1)
    m20, m10 = _build_masks()
    b0, b1 = _build_bands()
    ident = np.eye(128, dtype=np.float32)
    w1 = np.asarray(inputs["w1"], np.float32)
    common = {
        "x_full_b": xb,
        "bands0": b0.astype(bfloat16), "bands1": b1.astype(bfloat16),
        "mask20": m20, "mask10": m10, "ident": ident,
        "w1a": np.ascontiguousarray(w1[0:128]),
        "w1b": np.ascontiguousarray(w1[128:256]),
        "b1": np.asarray(inputs["b1"], np.float32).reshape(128, 1),
        "gamma": np.asarray(inputs["gamma"], np.float32).reshape(128, 1),
        "beta": np.asarray(inputs["beta"], np.float32).reshape(128, 1),
        "w2": np.asarray(inputs["w2"], np.float32),
        "b2": np.asarray(inputs["b2"], np.float32).reshape(64, 1),
        "w3": np.asarray(inputs["w3"], np.float32),
        "b3": np.asarray(inputs["b3"], np.float32).reshape(3, 1),
        "positions": pos,
        "xlast": np.ascontiguousarray(x[-1].reshape(128, 1)),
        "onehot127": np.eye(128, dtype=np.float32)[:, 127:128].copy(),
        "onehot2": np.array([[0.0], [0.0], [1.0]], np.float32),
    }
    in_maps = []
    for c in range(NC_N):
        rows = (c * CHUNK + np.arange(XROWS)) % T
        v20, h10, r10 = _core_masks(c)
        m = dict(common)
        xcb = np.ascontiguousarray(xb[rows])
        m["xchunk_b"] = xcb
        m["xT_b"] = np.ascontiguousarray(xcb.T)
        m["valid20"] = v20
        m["hist10"] = h10
        m["recent10"] = r10
        in_maps.append(m)
    return in_maps


def _combine(per_core):
    count20 = sum(float(per_core[c][0, S_COUNT20]) for c in range(NC_N))
    hist_raw = sum(float(per_core[c][0, S_HIST10]) for c in range(NC_N))
    rec_raw = sum(float(per_core[c][0, S_RECENT10]) for c in range(NC_N))
    cs_sum = sum(float(per_core[c][0, S_CSSUM]) for c in range(NC_N))
    ssq_sum = sum(float(per_core[c][0, S_SSQ]) for c in range(NC_N))
    cs_first = float(per_core[0][0, S_CSFIRST])
    cs_last = float(per_core[NC_N - 1][0, S_CSLAST])
    r0 = per_core[0][0]
    sum_corr = float(r0[S_SUMCORR])
    sum_abs = float(r0[S_SUMABS])
    trace_c = float(r0[S_TRACE])
    pa_sum = float(r0[S_PASUM])
    pa_max = float(r0[S_PAMAX])
    severity = float(r0[S_SEV])
    T6, T9 = float(r0[S_T6]), float(r0[S_T9])

    phase_locking = count20 / N20
    nh = N10 - 5
    hist = (hist_raw - nh * A) * INV_OD / nh
    recent = (rec_raw - 5 * A) * INV_OD / 5.0
    surge = 0.0
    if hist > 0:
        surge = min(max((recent - hist) / hist, 0.0), 1.0)
    avg_disp = cs_sum / T
    trend = -(cs_last - cs_first) / (T - 1)
    herding_index = min(max(trend / (avg_disp + 1e-6) + 0.5, 0.0), 1.0)
    avg_corr = (sum_corr - trace_c) / (A * (A - 1))
    lam = np.exp((8.0 * np.log(T6) + np.log(T9)) / 512.0)
    sync_risk = min(1.0, (lam / A) * avg_corr)
    return_div = 1.0 - sum_abs / (A * A)
    pos_div = 1.0 - pa_max / pa_sum
    div_loss = 1.0 - np.sqrt(return_div * pos_div)
    avg_conc = (A * A / 2.0 + ssq_sum / (2.0 * T) - A) / (A * (A - 1))
    phase_coupling = min(max((avg_conc - 0.5) * 2.0, 0.0), 1.0)
    collective = (herding_index + sync_risk + div_loss) / 3.0
    return np.array([herding_index, severity, sync_risk, phase_locking,
                     div_loss, surge, phase_coupling, collective],
                    dtype=np.float32)


def _ensure_ntff_hook():
    """Install the axon NTFF profile hook if the image lacks antenv.axon_hooks."""
    import sys
    import types
    try:
        import antenv.axon_hooks  # noqa: F401
        return True
    except ImportError:
        pass
    try:
        import antenv
        from trn_agent_boot.trn_boot import _ntff_profile_via_ctypes
        mod = types.ModuleType("antenv.axon_hooks")
        state = {}
        mod.set_axon_ntff_profile_hook = lambda h: state.update(h=h)
        mod.get_axon_ntff_profile_hook = lambda: state.get("h")
        sys.modules["antenv.axon_hooks"] = mod
        antenv.axon_hooks = mod
        hook = _ntff_profile_via_ctypes("/opt/axon/libaxon_pjrt.so")
        mod.set_axon_ntff_profile_hook(hook)
        return hook is not None
    except Exception:
        return False


def _run(inputs, trace=False):
    from concourse.bass_utils import run_bass_kernel_spmd
    if trace:
        trace = _ensure_ntff_hook()
    if "nc" not in _PLAN:
        _PLAN["nc"] = _build_program()
    nc = _PLAN["nc"]
    in_maps = _prep_in_maps(inputs)
    res = run_bass_kernel_spmd(nc, in_maps, core_ids=list(range(NC_N)),
                               trace=trace)
    per_core = [res.results[c]["out_vec"] for c in range(NC_N)]
    return _combine(per_core), res


def kernel(**inputs) -> np.ndarray:
    out, _ = _run(inputs, trace=False)
    return out


# revision 13
# speedup vs baseline: 2.1053x; 1.2038x over previous
"""Trainium2 Bass kernel for nn_EmergentRiskMetrics.

Contract: kernel(**inputs) takes the FULL unsharded inputs (as produced by
setup_inputs()) and returns the FULL output (shape [8], float32).

Sharding: data-parallel over the time axis. Each of the 8 cores owns 1024
contiguous window starts (plus a 128-row halo) for the two rolling-window
correlation scans; the sign-concordance partial sum and cross-sectional
stds are computed on the owning core and combined as scalars on the host.
The full-T covariance (needed on-device for the eigenvalue iteration) is
replicated: every core re-computes X^T X from bf16 tiles of the full
sequence (~1 us of 128^3 bf16 matmuls + ~2 MB of DMA, overlapped with the
rolling phase). An AllReduce-based variant was measured at ~66 us of pure
collective latency for 64 KB on this runtime — replication is far cheaper.

DMA-trigger serialization dominates small-tensor staging, so the host
packs every fp32 constant (masks, identity, MLP weights, positions, ...)
into ONE [128,1024] tensor, and the bf16 bands + pre-transposed chunk
into ONE [128,1664] tensor; x_full lands via 8 big strided DMAs split
across the two HWDGE queues (sync + scalar).

Windowed sums are banded-matrix matmuls on the tensor engine (bands
pre-scaled by 1/sqrt(w) so the mean-correction q^2 term folds into the
V-mask), u = 1/std via reciprocal_approx_fast + Sqrt, and the whole
rolling path runs in bf16 (validated: all rolling-derived outputs have
orders-of-magnitude margin against bf16 noise; d2 >= 0 holds exactly
because S and P derive from the same bf16 x). V*mask+reduce is fused via
tensor_tensor_reduce.

Top eigenvalue: corr is squared 9 times in bf16 (fp32 PSUM accumulate);
traces at step 6 (normalization) and step 9 give lam = (T9*T6^8)^(1/512)
on the host.

Device outputs are per-core partial scalars; the host only gathers them
(sums partial sums, applies the final scalar clips/divides) to assemble
the 8 outputs.
"""

import numpy as np

T = 8192
A = 128
W20 = 20
W10 = 10
NC_N = 8
CHUNK = 1024            # window starts per core
XROWS = 1152            # rows of per-core chunk (9 x 128, incl. halo)
NBLK = XROWS // 128     # 9
R20 = 128 + W20 - 1     # 147
R10 = 128 + W10 - 1     # 137
N20 = T - W20           # 8172 rolling-20 windows
N10 = T - W10           # 8182 rolling-10 windows
OUT_SLOTS = 24
INV_OD = 1.0 / (A * (A - 1))
# rolling20 > 0.7 in corr units == raw quadratic sum > this
THRESH20 = 0.7 * (A * (A - 1)) + A

S_COUNT20, S_HIST10, S_RECENT10, S_CSSUM, S_CSFIRST, S_CSLAST, \
    S_SUMCORR, S_SUMABS, S_TRACE, S_PASUM, S_PAMAX, S_SEV, S_SSQ, \
    S_T6, S_T9 = range(15)

# packed fp32 constant tensor column layout
CP_IDENT = 0
CP_M20 = 128                 # 275 cols
CP_M10 = CP_M20 + R20 + 128  # 403, 265 cols
CP_V20 = CP_M10 + R10 + 128  # 668
CP_H10 = CP_V20 + 8
CP_R10 = CP_H10 + 8
CP_W1A = CP_R10 + 8          # 692
CP_W1B = CP_W1A + 128        # 820
CP_B1 = CP_W1B + 128         # 948
CP_GAM = CP_B1 + 1
CP_BET = CP_GAM + 1
CP_W2 = CP_BET + 1           # 951, 64 cols
CP_B2 = CP_W2 + 64           # 1015
CP_W3 = CP_B2 + 1            # 1016, 3 cols
CP_B3 = CP_W3 + 3            # 1019
CP_OH2 = CP_B3 + 1
CP_OH127 = CP_OH2 + 1
CP_POS = CP_OH127 + 1
CP_XLAST = CP_POS + 1
CP_N = CP_XLAST + 1          # 1024

BP_B0 = 0
BP_B1 = 256
BP_XT = 512
BP_N = BP_XT + XROWS         # 1664

_PLAN = {}


def _build_masks():
    # V-masks: +1 over the window band, -1 on the q^2 column
    m20 = np.zeros((128, R20 + 128), np.float32)
    m10 = np.zeros((128, R10 + 128), np.float32)
    for j in range(128):
        m20[j, j:j + W20] = 1.0
        m20[j, R20 + j] = -1.0
        m10[j, j:j + W10] = 1.0
        m10[j, R10 + j] = -1.0
    return m20, m10


def _build_bands():
    # bands0/1 [128 t, 256]: cols 0:128 window-20 (scaled 1/sqrt20),
    # cols 128:256 window-10 (scaled 1/sqrt10). S' = B0^T x_k + B1^T x_{k+1}
    b0 = np.zeros((128, 256), np.float32)
    b1 = np.zeros((128, 256), np.float32)
    s20 = 1.0 / np.sqrt(W20)
    s10 = 1.0 / np.sqrt(W10)
    for j in range(128):
        b0[j:min(128, j + W20), j] = s20
        if j + W20 > 128:
            b1[0:j + W20 - 128, j] = s20
        b0[j:min(128, j + W10), 128 + j] = s10
        if j + W10 > 128:
            b1[0:j + W10 - 128, 128 + j] = s10
    return b0, b1


def _core_masks(c):
    g = c * CHUNK + np.arange(CHUNK)
    valid20 = (g < N20).astype(np.float32)
    hist10 = (g < N10 - 5).astype(np.float32)
    recent10 = ((g >= N10 - 5) & (g < N10)).astype(np.float32)
    # device layout [128 partitions (j in chunk), 8 chunk-columns]
    return (np.ascontiguousarray(valid20.reshape(8, 128).T),
            np.ascontiguousarray(hist10.reshape(8, 128).T),
            np.ascontiguousarray(recent10.reshape(8, 128).T))


def _build_program():
    import os
    import concourse.bacc as bacc
    import concourse.tile as tile
    from concourse import mybir

    kbits = int(os.environ.get("KBITS", "63"))
    bigdma = int(os.environ.get("BIGDMA", "1"))
    use_ttr = int(os.environ.get("TTR", "1"))
    DO_ROLL = kbits & 1
    DO_CS = kbits & 2
    DO_COV = kbits & 4
    DO_EIG = kbits & 8
    DO_POS = kbits & 16
    DO_MLP = kbits & 32

    f32 = mybir.dt.float32
    bf16 = mybir.dt.bfloat16
    ALU = mybir.AluOpType
    ACT = mybir.ActivationFunctionType
    AX = mybir.AxisListType

    nc = bacc.Bacc("TRN2", target_bir_lowering=False, debug=False,
                   num_devices=NC_N)

    def din(name, shape, dt=f32):
        return nc.dram_tensor(name, shape, dt, kind="ExternalInput").ap()

    # partition-major layouts (host pre-permuted): col block i of x_full_pm
    # is x[i*128:(i+1)*128, :] with time-on-partitions — plain contiguous
    # DMAs with one descriptor per partition.
    x_full_pm = din("x_full_pm", [128, 64 * 128], bf16)
    xchunk_pm = din("xchunk_pm", [128, XROWS], bf16)
    cpack_in = din("cpack", [128, CP_N])
    bpack_in = din("bpack", [128, BP_N], bf16)
    out_d = nc.dram_tensor("out_vec", [1, OUT_SLOTS], f32,
                           kind="ExternalOutput").ap()

    with tile.TileContext(nc) as tc:
        with tc.tile_pool(name="const", bufs=1) as cst, \
             tc.tile_pool(name="persist", bufs=1) as per, \
             tc.tile_pool(name="sgs", bufs=3) as sgs, \
             tc.tile_pool(name="wrk", bufs=3) as wrk, \
             tc.tile_pool(name="small", bufs=6) as sml, \
             tc.tile_pool(name="ps", bufs=1, space="PSUM") as ps:

            psum_bufs = {"covq": 1, "band": 2, "zp": 2, "big": 1, "sc": 2}

            def psum(shape, tag):
                return ps.tile(shape, f32, tag=tag, name=tag,
                               bufs=psum_bufs[tag])

            # ---- packed loads: 3 plain DMAs on sync for all staging ----
            bpk = cst.tile([128, BP_N], bf16, tag="bpk")
            nc.sync.dma_start(bpk[:], bpack_in[:, :])
            xck = per.tile([128, XROWS], bf16, tag="xck")
            nc.sync.dma_start(xck[:], xchunk_pm[:, :])
            cpk = cst.tile([128, CP_N], f32, tag="cpk")
            nc.sync.dma_start(cpk[:], cpack_in[:, :])

            b0b = bpk[:, BP_B0:BP_B0 + 256]
            b1b = bpk[:, BP_B1:BP_B1 + 256]
            xTb = bpk[:, BP_XT:BP_XT + XROWS]
            xcbs = [xck[:, j * 128:(j + 1) * 128] for j in range(NBLK)]

            ident = cpk[:, CP_IDENT:CP_IDENT + 128]
            m20 = cpk[:, CP_M20:CP_M20 + R20 + 128]
            m10 = cpk[:, CP_M10:CP_M10 + R10 + 128]
            v20 = cpk[:, CP_V20:CP_V20 + 8]
            h10 = cpk[:, CP_H10:CP_H10 + 8]
            r10 = cpk[:, CP_R10:CP_R10 + 8]
            w1a = cpk[:, CP_W1A:CP_W1A + 128]
            w1b = cpk[:, CP_W1B:CP_W1B + 128]
            b1 = cpk[:, CP_B1:CP_B1 + 1]
            gam = cpk[:, CP_GAM:CP_GAM + 1]
            bet = cpk[:, CP_BET:CP_BET + 1]
            w2 = cpk[:, CP_W2:CP_W2 + 64]
            b2 = cpk[0:64, CP_B2:CP_B2 + 1]
            w3 = cpk[0:64, CP_W3:CP_W3 + 3]
            b3 = cpk[0:3, CP_B3:CP_B3 + 1]
            oh2 = cpk[0:3, CP_OH2:CP_OH2 + 1]
            oh127 = cpk[:, CP_OH127:CP_OH127 + 1]
            pos_sb = cpk[:, CP_POS:CP_POS + 1]
            xl = cpk[:, CP_XLAST:CP_XLAST + 1]

            ones = cst.tile([128, 1], f32, tag="ones")
            nc.vector.memset(ones[:], 1.0)
            ones_row = cst.tile([1, 128], f32, tag="ones_row")
            nc.vector.memset(ones_row[:], 1.0)

            out_sb = per.tile([1, OUT_SLOTS], f32, tag="out_sb")
            nc.vector.memset(out_sb[:], 0.0)

            def slot(i):
                return out_sb[:, i:i + 1]

            def psum_scalar(vec_sb, p=128):
                o = psum([1, 1], "sc")
                lhs = ones[0:p, :] if p != 128 else ones[:]
                nc.tensor.matmul(o[:], lhsT=lhs, rhs=vec_sb,
                                 start=True, stop=True, skip_group_check=True)
                return o

            # ---- full x for replicated cov: 2 halves on the 2 HWDGE queues
            xfp = per.tile([128, 64 * 128], bf16, tag="xfp")
            if DO_COV:
                if bigdma:
                    nc.sync.dma_start(xfp[:, 0:4096], x_full_pm[:, 0:4096])
                    nc.scalar.dma_start(xfp[:, 4096:8192],
                                        x_full_pm[:, 4096:8192])
                else:
                    for i in range(8):
                        eng = nc.sync if i < 4 else nc.scalar
                        eng.dma_start(xfp[:, i * 1024:(i + 1) * 1024],
                                      x_full_pm[:, i * 1024:(i + 1) * 1024])

            # ---- per-tile squares (gpsimd; reads SBUF only) ----
            xsqbs = []
            for j in range(NBLK):
                xsqb = per.tile([128, 128], bf16, tag="xsqb%d" % j)
                nc.gpsimd.tensor_mul(xsqb[:], xcbs[j], xcbs[j])
                xsqbs.append(xsqb)

            # ---- sharded sign concordance ----
            mq = psum([128, 128], "big")
            for i in range(8):
                sg = sgs.tile([128, 128], bf16, tag="sg")
                nc.scalar.activation(sg[:], xcbs[i], ACT.Sign)
                nc.tensor.matmul(mq[:], lhsT=sg[:], rhs=sg[:],
                                 start=(i == 0), stop=(i == 7),
                                 skip_group_check=True)
            mr = sml.tile([128, 1], f32, tag="mr")
            nc.vector.tensor_reduce(mr[:], mq[:], axis=AX.X, op=ALU.add)
            nc.vector.tensor_copy(slot(S_SSQ), psum_scalar(mr[:])[:])

            # ---- cross-sectional sums (independent; fills startup) ----
            if DO_CS:
                cs_s = per.tile([128, 8], f32, tag="cs_s")
                cs_q = per.tile([128, 8], f32, tag="cs_q")
                for b in range(8):
                    nc.vector.tensor_reduce(cs_s[:, b:b + 1], xcbs[b],
                                            axis=AX.X, op=ALU.add)
                    nc.vector.tensor_reduce(cs_q[:, b:b + 1], xsqbs[b][:],
                                            axis=AX.X, op=ALU.add)

            # ================= rolling windows + cov, interleaved ==========
            covq = psum([128, 128], "covq")
            num20 = per.tile([128, 8], f32, tag="num20")
            num10 = per.tile([128, 8], f32, tag="num10")
            for k in range(8):
                if DO_ROLL:
                    sp_ = psum([128, 256], "band")
                    nc.tensor.matmul(sp_[:], lhsT=xcbs[k], rhs=b0b,
                                     start=True, stop=False,
                                     skip_group_check=True)
                    nc.tensor.matmul(sp_[:], lhsT=xcbs[k + 1], rhs=b1b,
                                     start=False, stop=True,
                                     skip_group_check=True)
                    pp = psum([128, 256], "band")
                    nc.tensor.matmul(pp[:], lhsT=xsqbs[k][:], rhs=b0b,
                                     start=True, stop=False,
                                     skip_group_check=True)
                    nc.tensor.matmul(pp[:], lhsT=xsqbs[k + 1][:], rhs=b1b,
                                     start=False, stop=True,
                                     skip_group_check=True)
                    # d2 = P - S^2/w = pp*sqrt(w) - S'^2 (>=0 exactly)
                    sq = wrk.tile([128, 256], f32, tag="sq")
                    nc.scalar.activation(sq[:], sp_[:], ACT.Square)
                    d2 = wrk.tile([128, 256], f32, tag="d2")
                    nc.vector.scalar_tensor_tensor(
                        d2[:, 0:128], in0=pp[:, 0:128],
                        scalar=float(np.sqrt(W20)),
                        in1=sq[:, 0:128], op0=ALU.mult, op1=ALU.subtract)
                    nc.vector.scalar_tensor_tensor(
                        d2[:, 128:256], in0=pp[:, 128:256],
                        scalar=float(np.sqrt(W10)),
                        in1=sq[:, 128:256], op0=ALU.mult, op1=ALU.subtract)
                    rd2 = wrk.tile([128, 256], f32, tag="rd2")
                    nc.vector.reciprocal_approx_fast(rd2[:], d2[:])
                    ub = wrk.tile([128, 256], bf16, tag="ub")
                    nc.scalar.activation(ub[:], rd2[:], ACT.Sqrt)
                    spb = wrk.tile([128, 256], bf16, tag="spb")
                    nc.vector.tensor_copy(spb[:], sp_[:])

                    for (wi, R, msk) in ((0, R20, m20), (1, R10, m10)):
                        zp = psum([128, R + 128], "zp")
                        nc.tensor.matmul(
                            zp[:, 0:R], lhsT=ub[:, wi * 128:(wi + 1) * 128],
                            rhs=xTb[:, k * 128:k * 128 + R],
                            start=True, stop=True, skip_group_check=True)
                        nc.tensor.matmul(
                            zp[:, R:R + 128],
                            lhsT=ub[:, wi * 128:(wi + 1) * 128],
                            rhs=spb[:, wi * 128:(wi + 1) * 128],
                            start=True, stop=True, skip_group_check=True)
                        V = wrk.tile([128, R + 128], f32, tag="V%d" % wi)
                        nc.scalar.activation(V[:], zp[:], ACT.Square)
                        dst = num20 if wi == 0 else num10
                        if use_ttr:
                            scr = wrk.tile([128, R + 128], f32,
                                           tag="scr%d" % wi)
                            nc.vector.tensor_tensor_reduce(
                                scr[:], V[:], msk, scale=1.0, scalar=0.0,
                                op0=ALU.mult, op1=ALU.add,
                                accum_out=dst[:, k:k + 1])
                        else:
                            scr = wrk.tile([128, R + 128], f32,
                                           tag="scr%d" % wi)
                            nc.gpsimd.tensor_mul(scr[:], V[:], msk)
                            nc.vector.tensor_reduce(dst[:, k:k + 1], scr[:],
                                                    axis=AX.X, op=ALU.add)
                # 8 of the 64 replicated cov matmuls per rolling chunk
                if DO_COV:
                    for i in range(8):
                        t_ = xfp[:, (k * 8 + i) * 128:(k * 8 + i + 1) * 128]
                        nc.tensor.matmul(covq[:], lhsT=t_, rhs=t_,
                                         start=(k == 0 and i == 0),
                                         stop=(k == 7 and i == 7),
                                         skip_group_check=True)

            if DO_ROLL:
                # phase locking count: num20 > thresh, masked valid
                cmp = sml.tile([128, 8], f32, tag="cmp")
                nc.vector.tensor_scalar(cmp[:], num20[:], THRESH20, None,
                                        ALU.is_gt)
                cmp2 = sml.tile([128, 8], f32, tag="cmp2")
                nc.gpsimd.tensor_mul(cmp2[:], cmp[:], v20)
                cnt = sml.tile([128, 1], f32, tag="cnt")
                nc.vector.tensor_reduce(cnt[:], cmp2[:], axis=AX.X,
                                        op=ALU.add)
                nc.vector.tensor_copy(slot(S_COUNT20), psum_scalar(cnt[:])[:])
                hv = sml.tile([128, 8], f32, tag="hv")
                nc.gpsimd.tensor_mul(hv[:], num10[:], h10)
                hs = sml.tile([128, 1], f32, tag="hs")
                nc.vector.tensor_reduce(hs[:], hv[:], axis=AX.X, op=ALU.add)
                nc.vector.tensor_copy(slot(S_HIST10), psum_scalar(hs[:])[:])
                rv = sml.tile([128, 8], f32, tag="rv")
                nc.gpsimd.tensor_mul(rv[:], num10[:], r10)
                rs = sml.tile([128, 1], f32, tag="rs")
                nc.vector.tensor_reduce(rs[:], rv[:], axis=AX.X, op=ALU.add)
                nc.vector.tensor_copy(slot(S_RECENT10), psum_scalar(rs[:])[:])

            # ---- cross-sectional std finish ----
            if DO_CS:
                cs_sq = sml.tile([128, 8], f32, tag="cs_sq")
                nc.scalar.activation(cs_sq[:], cs_s[:], ACT.Square)
                cs_var = sml.tile([128, 8], f32, tag="cs_var")
                nc.vector.scalar_tensor_tensor(
                    cs_var[:], in0=cs_sq[:], scalar=-1.0 / A, in1=cs_q[:],
                    op0=ALU.mult, op1=ALU.add)
                csstd = per.tile([128, 8], f32, tag="csstd")
                nc.scalar.activation(csstd[:], cs_var[:], ACT.Sqrt,
                                     scale=1.0 / (A - 1))
                csr = sml.tile([128, 1], f32, tag="csr")
                nc.vector.tensor_reduce(csr[:], csstd[:], axis=AX.X,
                                        op=ALU.add)
                nc.vector.tensor_copy(slot(S_CSSUM), psum_scalar(csr[:])[:])
                nc.vector.tensor_copy(slot(S_CSFIRST), csstd[0:1, 0:1])
                cslast_p = psum([1, 1], "sc")
                nc.tensor.matmul(cslast_p[:], lhsT=oh127, rhs=csstd[:, 7:8],
                                 start=True, stop=True, skip_group_check=True)
                nc.vector.tensor_copy(slot(S_CSLAST), cslast_p[:])

            # ================= position diversity =================
            if DO_POS:
                pa = per.tile([128, 1], f32, tag="pa")
                nc.scalar.activation(pa[:], pos_sb, ACT.Abs)
                nc.vector.tensor_copy(slot(S_PASUM), psum_scalar(pa[:])[:])
                paT_p = psum([1, 128], "sc")
                nc.tensor.transpose(paT_p[:], pa[:], ident)
                paT = sml.tile([1, 128], f32, tag="paT")
                nc.vector.tensor_copy(paT[:], paT_p[:])
                nc.vector.tensor_reduce(slot(S_PAMAX), paT[:], axis=AX.X,
                                        op=ALU.max)

            # ================= herding MLP =================
            if DO_MLP:
                h1p = psum([128, 1], "sc")
                nc.tensor.matmul(h1p[:], lhsT=w1a, rhs=xl, start=True,
                                 stop=False, skip_group_check=True)
                nc.tensor.matmul(h1p[:], lhsT=w1b, rhs=pos_sb,
                                 start=False, stop=True,
                                 skip_group_check=True)
                h1 = sml.tile([128, 1], f32, tag="h1")
                nc.scalar.activation(h1[:], h1p[:], ACT.Relu, bias=b1)
                gk = sml.tile([128, 1], f32, tag="gk")
                nc.vector.tensor_scalar(gk[:], gam,
                                        float(1.0 / np.sqrt(1.0 + 1e-5)),
                                        None, ALU.mult)
                h1b = sml.tile([128, 1], f32, tag="h1b")
                nc.vector.tensor_scalar(h1b[:], h1[:], gk[:], bet,
                                        ALU.mult, ALU.add)
                h2p = psum([64, 1], "sc")
                nc.tensor.matmul(h2p[:], lhsT=w2, rhs=h1b[:], start=True,
                                 stop=True, skip_group_check=True)
                h2 = sml.tile([64, 1], f32, tag="h2")
                nc.scalar.activation(h2[:], h2p[:], ACT.Relu, bias=b2)
                lg = psum([3, 1], "sc")
                nc.tensor.matmul(lg[:], lhsT=w3, rhs=h2[:], start=True,
                                 stop=True, skip_group_check=True)
                exps = sml.tile([3, 1], f32, tag="exps")
                nc.scalar.activation(exps[:], lg[:], ACT.Exp, bias=b3)
                esum = psum_scalar(exps[:], p=3)
                esum_sb = sml.tile([1, 1], f32, tag="esum_sb")
                nc.vector.tensor_copy(esum_sb[:], esum[:])
                erec = sml.tile([1, 1], f32, tag="erec")
                nc.vector.reciprocal(erec[:], esum_sb[:])
                e2p = psum([1, 1], "sc")
                nc.tensor.matmul(e2p[:], lhsT=oh2, rhs=exps[:], start=True,
                                 stop=True, skip_group_check=True)
                e2_sb = sml.tile([1, 1], f32, tag="e2_sb")
                nc.vector.tensor_copy(e2_sb[:], e2p[:])
                nc.vector.tensor_mul(slot(S_SEV), e2_sb[:], erec[:])

            # ================= cov postprocessing + eigenvalue =============
            if DO_COV:
                cov = per.tile([128, 128], f32, tag="cov")
                nc.scalar.activation(cov[:], covq[:], ACT.Copy)
                dscr = wrk.tile([128, 128], f32, tag="dscr")
                nc.vector.tensor_mul(dscr[:], cov[:], ident)
                diag = per.tile([128, 1], f32, tag="diag")
                nc.vector.tensor_reduce(diag[:], dscr[:], axis=AX.X,
                                        op=ALU.add)
                dstd = per.tile([128, 1], f32, tag="dstd")
                nc.scalar.activation(dstd[:], diag[:], ACT.Sqrt)
                ucol = per.tile([128, 1], f32, tag="ucol")
                nc.vector.reciprocal(ucol[:], dstd[:])
                u2 = sml.tile([128, 1], f32, tag="u2")
                nc.vector.tensor_mul(u2[:], ucol[:], ucol[:])
                du2 = sml.tile([128, 1], f32, tag="du2")
                nc.vector.tensor_mul(du2[:], u2[:], diag[:])
                nc.vector.tensor_copy(slot(S_TRACE), psum_scalar(du2[:])[:])

                uT_p = psum([1, 128], "sc")
                nc.tensor.transpose(uT_p[:], ucol[:], ident)
                uT = per.tile([1, 128], f32, tag="uT")
                nc.vector.tensor_copy(uT[:], uT_p[:])

                def quad_form(mat_sb, out_slot):
                    qr = psum([1, 128], "sc")
                    nc.tensor.matmul(qr[:], lhsT=ucol[:], rhs=mat_sb,
                                     start=True, stop=True,
                                     skip_group_check=True)
                    qscr = sml.tile([1, 128], f32, tag="qscr")
                    nc.vector.tensor_mul(qscr[:], qr[:], uT[:])
                    qacc = sml.tile([1, 1], f32, tag="qacc")
                    nc.vector.tensor_reduce(qacc[:], qscr[:], axis=AX.X,
                                            op=ALU.add)
                    nc.vector.tensor_copy(out_slot, qacc[:])

                quad_form(cov[:], slot(S_SUMCORR))
                acov = per.tile([128, 128], f32, tag="acov")
                nc.scalar.activation(acov[:], cov[:], ACT.Abs)
                quad_form(acov[:], slot(S_SUMABS))

                # corr = diag(u) cov diag(u) -> bf16
                brow = per.tile([128, 128], f32, tag="brow")
                nc.vector.tensor_scalar(brow[:], cov[:], ucol[:], None,
                                        ALU.mult)
                bt_p = psum([128, 128], "big")
                nc.tensor.transpose(bt_p[:], brow[:], ident)
                corr = per.tile([128, 128], bf16, tag="corr")
                nc.scalar.activation(corr[:], bt_p[:], ACT.Copy,
                                     scale=ucol[:])

            if DO_COV and DO_EIG:
                def trace_of(p, out_slot):
                    escr = wrk.tile([128, 128], f32, tag="escr")
                    nc.vector.tensor_mul(escr[:], p[:], ident)
                    edg = sml.tile([128, 1], f32, tag="edg")
                    nc.vector.tensor_reduce(edg[:], escr[:], axis=AX.X,
                                            op=ALU.add)
                    trp = psum_scalar(edg[:])
                    tr_sb = sml.tile([1, 1], f32, tag="tr_sb")
                    nc.vector.tensor_copy(tr_sb[:], trp[:])
                    nc.vector.tensor_copy(out_slot, tr_sb[:])
                    return tr_sb

                M = corr
                for kk in range(9):
                    p = psum([128, 128], "big")
                    nc.tensor.matmul(p[:], lhsT=M[:], rhs=M[:],
                                     start=True, stop=True,
                                     skip_group_check=True)
                    Mn = wrk.tile([128, 128], bf16, tag="Mn")
                    if kk == 5:
                        t6 = trace_of(p, slot(S_T6))
                        rcp1 = sml.tile([1, 1], f32, tag="rcp1")
                        nc.vector.reciprocal_approx_fast(rcp1[:], t6[:])
                        bc = psum([128, 1], "sc")
                        nc.tensor.matmul(bc[:], lhsT=ones_row[:], rhs=rcp1[:],
                                         start=True, stop=True,
                                         skip_group_check=True)
                        bcc = sml.tile([128, 1], f32, tag="bcc")
                        nc.vector.tensor_copy(bcc[:], bc[:])
                        nc.scalar.activation(Mn[:], p[:], ACT.Copy,
                                             scale=bcc[:])
                    elif kk == 8:
                        trace_of(p, slot(S_T9))
                        continue
                    else:
                        nc.scalar.activation(Mn[:], p[:], ACT.Copy)
                    M = Mn

            # ================= write out =================
            nc.sync.dma_start(out_d[:, :], out_sb[:])

    nc.compile()
    return nc


def _prep_in_maps(inputs):
    import ml_dtypes
    bfloat16 = ml_dtypes.bfloat16
    x = np.ascontiguousarray(np.asarray(inputs["returns_sequence"],
                                        dtype=np.float32))
    xb = x.astype(bfloat16)
    m20, m10 = _build_masks()
    b0, b1 = _build_bands()

    cpack = np.zeros((128, CP_N), np.float32)
    cpack[:, CP_IDENT:CP_IDENT + 128] = np.eye(128, dtype=np.float32)
    cpack[:, CP_M20:CP_M20 + R20 + 128] = m20
    cpack[:, CP_M10:CP_M10 + R10 + 128] = m10
    w1 = np.asarray(inputs["w1"], np.float32)
    cpack[:, CP_W1A:CP_W1A + 128] = w1[0:128]
    cpack[:, CP_W1B:CP_W1B + 128] = w1[128:256]
    cpack[:, CP_B1] = np.asarray(inputs["b1"], np.float32)
    cpack[:, CP_GAM] = np.asarray(inputs["gamma"], np.float32)
    cpack[:, CP_BET] = np.asarray(inputs["beta"], np.float32)
    cpack[:, CP_W2:CP_W2 + 64] = np.asarray(inputs["w2"], np.float32)
    cpack[0:64, CP_B2] = np.asarray(inputs["b2"], np.float32)
    cpack[0:64, CP_W3:CP_W3 + 3] = np.asarray(inputs["w3"], np.float32)
    cpack[0:3, CP_B3] = np.asarray(inputs["b3"], np.float32)
    cpack[2, CP_OH2] = 1.0
    cpack[127, CP_OH127] = 1.0
    cpack[:, CP_POS] = np.asarray(inputs["positions"], np.float32)
    cpack[:, CP_XLAST] = x[-1]

    # partition-major full x: col block i is rows [i*128,(i+1)*128)
    xfull_pm = np.ascontiguousarray(
        xb.reshape(64, 128, 128).transpose(1, 0, 2).reshape(128, 64 * 128))

    in_maps = []
    for c in range(NC_N):
        rows = (c * CHUNK + np.arange(XROWS)) % T
        v20, h10, r10 = _core_masks(c)
        cp = cpack.copy()
        cp[:, CP_V20:CP_V20 + 8] = v20
        cp[:, CP_H10:CP_H10 + 8] = h10
        cp[:, CP_R10:CP_R10 + 8] = r10
        xcb = np.ascontiguousarray(xb[rows])
        xchunk_pm = np.ascontiguousarray(
            xcb.reshape(NBLK, 128, 128).transpose(1, 0, 2)
            .reshape(128, XROWS))
        bpack = np.zeros((128, BP_N), bfloat16)
        bpack[:, BP_B0:BP_B0 + 256] = b0.astype(bfloat16)
        bpack[:, BP_B1:BP_B1 + 256] = b1.astype(bfloat16)
        bpack[:, BP_XT:BP_XT + XROWS] = xcb.T
        in_maps.append({
            "x_full_pm": xfull_pm,
            "xchunk_pm": xchunk_pm,
            "cpack": cp,
            "bpack": bpack,
        })
    return in_maps


def _combine(per_core):
    count20 = sum(float(per_core[c][0, S_COUNT20]) for c in range(NC_N))
    hist_raw = sum(float(per_core[c][0, S_HIST10]) for c in range(NC_N))
    rec_raw = sum(float(per_core[c][0, S_RECENT10]) for c in range(NC_N))
    cs_sum = sum(float(per_core[c][0, S_CSSUM]) for c in range(NC_N))
    ssq_sum = sum(float(per_core[c][0, S_SSQ]) for c in range(NC_N))
    cs_first = float(per_core[0][0, S_CSFIRST])
    cs_last = float(per_core[NC_N - 1][0, S_CSLAST])
    r0 = per_core[0][0]
    sum_corr = float(r0[S_SUMCORR])
    sum_abs = float(r0[S_SUMABS])
    trace_c = float(r0[S_TRACE])
    pa_sum = float(r0[S_PASUM])
    pa_max = float(r0[S_PAMAX])
    severity = float(r0[S_SEV])
    T6, T9 = float(r0[S_T6]), float(r0[S_T9])

    phase_locking = count20 / N20
    nh = N10 - 5
    hist = (hist_raw - nh * A) * INV_OD / nh
    recent = (rec_raw - 5 * A) * INV_OD / 5.0
    surge = 0.0
    if hist > 0:
        surge = min(max((recent - hist) / hist, 0.0), 1.0)
    avg_disp = cs_sum / T
    trend = -(cs_last - cs_first) / (T - 1)
    herding_index = min(max(trend / (avg_disp + 1e-6) + 0.5, 0.0), 1.0)
    avg_corr = (sum_corr - trace_c) / (A * (A - 1))
    lam = np.exp((8.0 * np.log(T6) + np.log(T9)) / 512.0)
    sync_risk = min(1.0, (lam / A) * avg_corr)
    return_div = 1.0 - sum_abs / (A * A)
    pos_div = 1.0 - pa_max / pa_sum
    div_loss = 1.0 - np.sqrt(return_div * pos_div)
    avg_conc = (A * A / 2.0 + ssq_sum / (2.0 * T) - A) / (A * (A - 1))
    phase_coupling = min(max((avg_conc - 0.5) * 2.0, 0.0), 1.0)
    collective = (herding_index + sync_risk + div_loss) / 3.0
    return np.array([herding_index, severity, sync_risk, phase_locking,
                     div_loss, surge, phase_coupling, collective],
                    dtype=np.float32)


def _ensure_ntff_hook():
    """Install the axon NTFF profile hook if the image lacks antenv.axon_hooks."""
    import sys
    import types
    try:
        import antenv.axon_hooks  # noqa: F401
        return True
    except ImportError:
        pass
    try:
        import antenv
        from trn_agent_boot.trn_boot import _ntff_profile_via_ctypes
        mod = types.ModuleType("antenv.axon_hooks")
        state = {}
        mod.set_axon_ntff_profile_hook = lambda h: state.update(h=h)
        mod.get_axon_ntff_profile_hook = lambda: state.get("h")
        sys.modules["antenv.axon_hooks"] = mod
        antenv.axon_hooks = mod
        hook = _ntff_profile_via_ctypes("/opt/axon/libaxon_pjrt.so")
        mod.set_axon_ntff_profile_hook(hook)
        return hook is not None
    except Exception:
        return False


def _run(inputs, trace=False):
    from concourse.bass_utils import run_bass_kernel_spmd
    if trace:
        trace = _ensure_ntff_hook()
    if "nc" not in _PLAN:
        _PLAN["nc"] = _build_program()
    nc = _PLAN["nc"]
    in_maps = _prep_in_maps(inputs)
    res = run_bass_kernel_spmd(nc, in_maps, core_ids=list(range(NC_N)),
                               trace=trace)
    per_core = [res.results[c]["out_vec"] for c in range(NC_N)]
    return _combine(per_core), res


def kernel(**inputs) -> np.ndarray:
    out, _ = _run(inputs, trace=False)
    return out
